# revision 37
# baseline (speedup 1.0000x reference)
"""Mistral attention (B=2, S=2048, H=4096, 32 q heads / 8 kv heads, rope) on
8 Trainium2 NeuronCores.

Sharding: DP=2 over batch x TP=4 over head groups. Core c handles batch
c//4 and q-heads 8g..8g+7 (kv heads 2g, 2g+1) where g = c%4. Attention is
fully local per core; each core produces a partial o_proj output
[2048, 4096] (bf16, contribution of its head group) and the host sums the
four TP partials per batch in fp32.

Mixed precision (validated on hw against the fp32 reference, relmax ~3e-3):
  - All projection matmuls (Q/K/V, o_proj) run as fp8-e4m3 DoubleRow
    matmuls (contraction 256/instr at 0.5 cycles/row) with a 3-term
    residual expansion  x_hi*w_hi + x_hi*w_lo + x_lo*w_hi  where
    t_hi = fp8(t), t_lo = fp8(t - t_hi).  X and all weights are split on
    the host; the o_proj input Y is split on-device during softmax
    normalization.  Weights are pre-scaled by 64 so fp8 quantization of
    W*64 stays in the normal range; the 64s cancel via the exp scale, the
    64-valued ones-vector in the denominator reduction, and a 1/64 scale
    on the output store.
  - Attention stays >= bf16 precision: scores fp16 x fp16, P bf16,
    PV bf16 x bf16, softmax denominator accumulated in bf16 (DVE 2x).
    The reciprocal and its PE-broadcast are bf16: denominators reach ~1e7
    so the reciprocal needs bf16's exponent range (fp16 underflows).
  - RoPE in [d, t] layout: 4 DVE ops reading the fp32 projection PSUM
    directly (walrus allows partition-shifted reads only for PSUM inputs)
    with host tables cosF=[cos;cos], snF=[-sin;+sin], fp16 out.

Schedule: PE engines execute in program order, so the program is laid out
to keep PE saturated:
  - Phase 1 streams X strips + per-output weights (all DMAs are
    per-partition contiguous, >=4KB segments, issued on SP in consumption
    order with double-buffered pools).
  - The q-strip-0 attention (ACT-heavy, little PE work) is hoisted into
    phase 1, one head after each strip-2/3 projection job, where the ACT
    engine is otherwise idle.  It uses a dedicated small PSUM pool set.
  - In phase 2, each remaining q-strip's attention interleaves the
    PREVIOUS strip's o_proj PSUM groups inside the ki loop (4 per head),
    giving PE filler while ACT exps drain.  Output stores batch 4
    n-chunks per DMA and issue on the idle GPSIMD queue.
  - Scores/exp tiles on the causal diagonal are restricted to their valid
    q-span; a single [128,128] lower-triangle mask handles the partial
    window.
"""
import sys

if "/opt/trn_rl_repo" not in sys.path:
    sys.path.insert(0, "/opt/trn_rl_repo")

import numpy as np
import ml_dtypes

BF16 = ml_dtypes.bfloat16
F8 = ml_dtypes.float8_e4m3

S = 2048          # sequence length per core
H = 4096          # hidden
D = 128           # head dim
TP = 4            # head-group shards
DP = 2            # batch shards
NHC = 8           # q heads per core
NKVC = 2          # kv heads per core
NPROJ = NHC + NKVC  # 10 projection outputs of 128 dims (8 q heads + 2 kv)
DQ = NHC * D      # 1024 q-proj out dims per core
DKV = NKVC * D    # 256 kv-proj out dims per core
KO = H // 128     # 32 contraction tiles for projections
NT = S // 128     # 16 t tiles
NSTRIP = S // 512  # 4 t strips
SCALE = 1.0 / np.sqrt(D)
ROPE_THETA = 10000.0
WSCALE = 64.0     # host premultiplier on all weights (fp8 range)

_CACHE = {}


def _mybir():
    import concourse.mybir as mybir
    return mybir


def _build_nc(phases=(1, 2, 3), repeats=1, tweaks=()):
    import concourse.mybir as mybir
    import concourse.tile as tile
    from concourse import bacc

    DT = mybir.dt
    DR = mybir.MatmulPerfMode.DoubleRow
    nc = bacc.Bacc(None, target_bir_lowering=False)

    # X strips, per-partition contiguous: [strip][p][ko*512]
    xh = nc.dram_tensor("xh", [NSTRIP, 128, KO * 512], DT.float8e4, kind="ExternalInput")
    xl = nc.dram_tensor("xl", [NSTRIP, 128, KO * 512], DT.float8e4, kind="ExternalInput")
    # Q+K weights, per-output contiguous: [out][p][ko*128]
    wh = nc.dram_tensor("wh", [NPROJ, 128, KO * 128], DT.float8e4, kind="ExternalInput")
    wl = nc.dram_tensor("wl", [NPROJ, 128, KO * 128], DT.float8e4, kind="ExternalInput")
    wvh = nc.dram_tensor("wvh", [128, KO * DKV], DT.float8e4, kind="ExternalInput")
    wvl = nc.dram_tensor("wvl", [128, KO * DKV], DT.float8e4, kind="ExternalInput")
    woh = nc.dram_tensor("woh", [128, NHC * H], DT.float8e4, kind="ExternalInput")
    wol = nc.dram_tensor("wol", [128, NHC * H], DT.float8e4, kind="ExternalInput")
    cosF = nc.dram_tensor("cosF", [D, S], DT.float16, kind="ExternalInput")
    snF = nc.dram_tensor("snF", [D, S], DT.float16, kind="ExternalInput")
    maskT = nc.dram_tensor("maskT", [128, 128], DT.bfloat16, kind="ExternalInput")
    outp = nc.dram_tensor("outp", [S, H], DT.bfloat16, kind="ExternalOutput")

    xh_r = xh.rearrange("s p (ko t) -> s p ko t", ko=KO)      # [4, 128, 32, 512]
    xl_r = xl.rearrange("s p (ko t) -> s p ko t", ko=KO)
    wh_r = wh.rearrange("o p (ko m) -> o p ko m", ko=KO)      # [10, 128, 32, 128]
    wl_r = wl.rearrange("o p (ko m) -> o p ko m", ko=KO)
    wvh_r = wvh.rearrange("p (ko m) -> p ko m", ko=KO)        # [128, 32, 256]
    wvl_r = wvl.rearrange("p (ko m) -> p ko m", ko=KO)
    woh_r = woh.rearrange("p (h n) -> p h n", h=NHC)          # [128, 8, 4096]
    wol_r = wol.rearrange("p (h n) -> p h n", h=NHC)

    EXP = mybir.ActivationFunctionType.Exp

    with tile.TileContext(nc) as tc:
        with tc.tile_pool(name="persist", bufs=1) as persist:
            qT = persist.tile([128, NHC, S], DT.float16)    # [d, head, t]
            kT = persist.tile([128, NKVC, S], DT.float16)   # [d, kv, t]
            vsb = persist.tile([128, NT, DKV], DT.bfloat16)  # [t%128, ttile, dv]
            yh = persist.tile([128, NHC, S], DT.float8e4)   # [d, head, q] hi
            yl = persist.tile([128, NHC, S], DT.float8e4)   # [d, head, q] lo
            mask_sb = persist.tile([128, 128], DT.bfloat16)
            ones = persist.tile([128, 1], DT.bfloat16)
            nc.vector.memset(ones[:], WSCALE)
            ones1 = persist.tile([1, 128], DT.bfloat16)
            nc.vector.memset(ones1[:], 1.0)

            def attn_head(qj, h, P):
                """One head's attention for q-strip qj: scores + exp + mask +
                denominator + PV + normalize + fp8 y split.  P = pool dict."""
                q0 = qj * 512
                nki = 4 * qj + 4
                kv = h // (NHC // NKVC)
                ps_cs = P["psCS"].tile([1, 512], DT.float32, tag="cs")
                ps_o = P["psO"].tile([128, 512], DT.float32, tag="o")
                acc = P["acc"].tile([128, 512], DT.bfloat16, tag="acc")
                for ki in range(nki):
                    k0 = ki * 128
                    r = ki - 4 * qj  # >=0 on the causal diagonal
                    v0 = max(r, 0) * 128  # valid q-span start
                    w_ = 512 - v0
                    ps_s = P["psS"].tile([128, 512], DT.float32, tag="s")
                    nc.tensor.matmul(
                        ps_s[:, 0:w_], kT[:, kv, k0:k0 + 128],
                        qT[:, h, q0 + v0:q0 + 512],
                        start=True, stop=True)
                    pt = P["pt"].tile([128, 512], DT.bfloat16, tag="pt")
                    nc.scalar.activation(pt[:, 0:w_], ps_s[:, 0:w_], EXP,
                                         scale=SCALE / (WSCALE * WSCALE))
                    if r >= 0:
                        # triangular window: cols [v0, v0+128)
                        nc.vector.tensor_mul(pt[:, 0:128], pt[:, 0:128],
                                             mask_sb[:])
                    if ki == 0:
                        nc.vector.tensor_copy(acc[:], pt[:])
                    else:
                        nc.vector.tensor_add(acc[:, v0:512], acc[:, v0:512],
                                             pt[:, 0:w_])
                    nc.tensor.matmul(
                        ps_o[:, v0:512],
                        vsb[:, ki, kv * 128:(kv + 1) * 128], pt[:, 0:w_],
                        start=(ki == 0), stop=(ki == nki - 1))
                    yield ki  # interleave point for the caller
                # denominator: 64 * sum_k pt via ones-matmul
                nc.tensor.matmul(ps_cs[:], ones[:], acc[:], start=True,
                                 stop=True)
                # recip values go down to ~1e-8 (big exp sums): needs bf16's
                # exponent range (fp16 underflows)
                recip = P["p2r"].tile([1, 512], DT.bfloat16, tag="recip")
                with nc.allow_low_precision(reason="bf16 softmax recip"):
                    nc.vector.reciprocal(recip[:], ps_cs[:])
                # broadcast recip across partitions via K=1 matmul
                ps_bc = P["psBC"].tile([128, 512], DT.float32, tag="bc")
                nc.tensor.matmul(ps_bc[:], ones1[:], recip[:], start=True,
                                 stop=True)
                # early PSUM evac in fp32 (ps_o is the UNnormalized sum of
                # exps -- huge; fp16 would overflow), then normalize + split
                yun = P["p2y"].tile([128, 512], DT.float32, tag="yun")
                nc.vector.tensor_copy(yun[:], ps_o[:])
                ytmp = P["p2y"].tile([128, 512], DT.float16, tag="ytmp")
                nc.vector.tensor_mul(ytmp[:], yun[:], ps_bc[:])
                nc.scalar.copy(yh[:, h, q0:q0 + 512], ytmp[:])
                with nc.allow_low_precision(reason="fp8 residual"):
                    nc.vector.tensor_sub(yl[:, h, q0:q0 + 512], ytmp[:],
                                         yh[:, h, q0:q0 + 512])

            for _rep in range(repeats):
                # ---------------- Phase 1: projections + rope ----------------
                with tc.tile_pool(name="p1c", bufs=1) as p1c, \
                     tc.tile_pool(name="p1x", bufs=2) as p1x, \
                     tc.tile_pool(name="p1w", bufs=2) as p1w, \
                     tc.tile_pool(name="p1t", bufs=1) as p1t, \
                     tc.tile_pool(name="q0pt", bufs=8) as q0pt, \
                     tc.tile_pool(name="q0acc", bufs=2) as q0acc, \
                     tc.tile_pool(name="q0r", bufs=2) as q0r, \
                     tc.tile_pool(name="q0y", bufs=2) as q0y, \
                     tc.tile_pool(name="p1ps", bufs=2, space="PSUM") as p1ps, \
                     tc.tile_pool(name="p1pv", bufs=1, space="PSUM") as p1pv, \
                     tc.tile_pool(name="q0S", bufs=2, space="PSUM") as q0S, \
                     tc.tile_pool(name="q0CS", bufs=1, space="PSUM") as q0CS, \
                     tc.tile_pool(name="q0BC", bufs=1, space="PSUM") as q0BC, \
                     tc.tile_pool(name="q0O", bufs=1, space="PSUM") as q0O:
                    q0pools = dict(psS=q0S, psCS=q0CS, psBC=q0BC, psO=q0O,
                                   pt=q0pt, acc=q0acc, p2r=q0r, p2y=q0y)
                    cos_sb = p1c.tile([D, S], DT.float16)
                    sn_sb = p1c.tile([D, S], DT.float16)
                    wvh_sb = p1c.tile([128, KO, DKV], DT.float8e4)
                    wvl_sb = p1c.tile([128, KO, DKV], DT.float8e4)

                    def rope(dst, ps, t0):
                        # dst [128, 512] fp16 (partition = d within head);
                        # rotate_half across the partition halves:
                        #   dst = ps*cosF + swap64(ps)*snF
                        # with cosF = [cos;cos], snF = [-sin;+sin].  The
                        # partition-shifted reads keep ps in PSUM (walrus
                        # forbids mismatched base partitions for SBUF+SBUF
                        # TensorTensor inputs, but PSUM+SBUF is fine).
                        u = p1t.tile([128, 512], DT.float16, tag="ru")
                        c = p1t.tile([128, 512], DT.float16, tag="rc")
                        nc.vector.tensor_mul(u[0:64, :], ps[64:128, :],
                                             sn_sb[0:64, t0:t0 + 512])
                        nc.vector.tensor_mul(u[64:128, :], ps[0:64, :],
                                             sn_sb[64:128, t0:t0 + 512])
                        nc.vector.tensor_mul(c[:], ps[:],
                                             cos_sb[:, t0:t0 + 512])
                        nc.vector.tensor_add(dst[:], c[:], u[:])

                    def load_w(h):
                        wth = p1w.tile([128, KO, 128], DT.float8e4, tag="wh")
                        wtl = p1w.tile([128, KO, 128], DT.float8e4, tag="wl")
                        nc.sync.dma_start(wth[:], wh_r[h])
                        nc.sync.dma_start(wtl[:], wl_r[h])
                        return wth, wtl

                    def load_x(ts):
                        xsh = p1x.tile([128, KO, 512], DT.float8e4, tag="xsh")
                        xsl = p1x.tile([128, KO, 512], DT.float8e4, tag="xsl")
                        return xsh, xsl

                    def x_chunk(dst, src_r, ts, c, nchunks=4):
                        ko_per = KO // nchunks
                        k0 = c * ko_per
                        nc.sync.dma_start(
                            dst[:, k0:k0 + ko_per, :],
                            src_r[ts, :, k0:k0 + ko_per, :])

                    def proj_group(ps, wth, wtl, xsh, xsl):
                        # half-ktile blocks: early matmuls only need the
                        # first half of each X tensor (startup latency)
                        nmm = 3 * (KO // 2)
                        i = 0
                        for jblk in range(2):
                            for wt, xt in ((wth, xsh), (wtl, xsh), (wth, xsl)):
                                for j in range(jblk * (KO // 4),
                                               (jblk + 1) * (KO // 4)):
                                    nc.tensor.matmul(
                                        ps[:], wt[:, 2 * j:2 * j + 2, :],
                                        xt[:, 2 * j:2 * j + 2, :],
                                        start=(i == 0), stop=(i == nmm - 1),
                                        perf_mode=DR)
                                    i += 1

                    strips = list(range(NSTRIP if 1 in phases else 0))
                    xs_cur = None
                    q0_iter = [None]  # suspended attn_head generator

                    def pump_q0():
                        if q0_iter[0] is not None:
                            try:
                                next(q0_iter[0])
                            except StopIteration:
                                q0_iter[0] = None

                    for ts in strips:
                        t0 = ts * 512
                        if ts == 0:
                            # startup order follows consumption deadlines of
                            # the half-ktile-blocked matmul order below
                            wth0 = p1w.tile([128, KO, 128], DT.float8e4, tag="wh")
                            nc.sync.dma_start(wth0[:], wh_r[0])
                            xs_cur = load_x(0)
                            x_chunk(xs_cur[0], xh_r, 0, 0)
                            wtl0 = p1w.tile([128, KO, 128], DT.float8e4, tag="wl")
                            nc.sync.dma_start(wtl0[:], wl_r[0])
                            w_cur = (wth0, wtl0)
                            x_chunk(xs_cur[0], xh_r, 0, 1)
                            x_chunk(xs_cur[1], xl_r, 0, 0)
                            x_chunk(xs_cur[1], xl_r, 0, 1)
                            nc.sync.dma_start(cos_sb[:], cosF[:])
                            nc.sync.dma_start(sn_sb[:], snF[:])
                            x_chunk(xs_cur[0], xh_r, 0, 2)
                            x_chunk(xs_cur[0], xh_r, 0, 3)
                            x_chunk(xs_cur[1], xl_r, 0, 2)
                            x_chunk(xs_cur[1], xl_r, 0, 3)
                            nc.sync.dma_start(mask_sb[:], maskT[:])
                        xsh, xsl = xs_cur
                        xs_nxt = load_x(ts + 1) if ts + 1 in strips else None
                        # Q^T heads and K^T kv-heads ([d, t] layout)
                        for h in range(NPROJ):
                            wth, wtl = w_cur
                            # prefetch next output's weights (and, spread over
                            # the loop, this/next strip's X chunks + consts)
                            w_nxt = None
                            if h + 1 < NPROJ:
                                w_nxt = load_w(h + 1)
                            elif ts + 1 in strips:
                                w_nxt = load_w(0)
                            if ts == 0:
                                if h == 6:
                                    nc.sync.dma_start(wvh_sb[:], wvh_r[:])
                                elif h == 7:
                                    nc.sync.dma_start(wvl_sb[:], wvl_r[:])
                            if 2 <= h < 6 and xs_nxt is not None:
                                x_chunk(xs_nxt[0], xh_r, ts + 1, h - 2)
                            elif 6 <= h and xs_nxt is not None:
                                x_chunk(xs_nxt[1], xl_r, ts + 1, h - 6)
                            ps = p1ps.tile([128, 512], DT.float32, tag="proj")
                            proj_group(ps, wth, wtl, xsh, xsl)
                            if h < NHC:
                                rope(qT[:, h, t0:t0 + 512], ps, t0)
                            else:
                                rope(kT[:, h - NHC, t0:t0 + 512], ps, t0)
                            w_cur = w_nxt
                            # hoisted q-strip-0 attention: one head per
                            # strip-2/3 projection job (ACT is idle here)
                            if 2 in phases and 1 in phases:
                                if ts == 2 and h < NHC:
                                    q0_iter[0] = attn_head(0, h, q0pools)
                                for _ in range(5):
                                    pump_q0()
                        # V natural layout [t, dv]
                        for mt in range(4):
                            tsl = slice(mt * 128, (mt + 1) * 128)
                            psv = p1pv.tile([128, DKV], DT.float32, tag="pv")
                            nmm = 3 * (KO // 2)
                            i = 0
                            for xt, wt in ((xsh, wvh_sb), (xsh, wvl_sb), (xsl, wvh_sb)):
                                for j in range(KO // 2):
                                    nc.tensor.matmul(
                                        psv[:], xt[:, 2 * j:2 * j + 2, tsl],
                                        wt[:, 2 * j:2 * j + 2, :],
                                        start=(i == 0), stop=(i == nmm - 1),
                                        perf_mode=DR)
                                    i += 1
                            nc.scalar.copy(vsb[:, ts * 4 + mt, :], psv[:])
                            for _ in range(2):
                                pump_q0()
                        xs_cur = xs_nxt
                    # drain any remaining q-strip-0 attention
                    while q0_iter[0] is not None:
                        pump_q0()

                # ------- Phases 2+3 interleaved, strip-pipelined -------
                with tc.tile_pool(name="p2pt", bufs=12) as p2pt, \
                     tc.tile_pool(name="p2r", bufs=2) as p2r, \
                     tc.tile_pool(name="p2acc", bufs=2) as p2acc, \
                     tc.tile_pool(name="p2y", bufs=3) as p2y, \
                     tc.tile_pool(name="p3w", bufs=1) as p3w, \
                     tc.tile_pool(name="p3o", bufs=3) as p3o, \
                     tc.tile_pool(name="psS", bufs=3, space="PSUM") as psS, \
                     tc.tile_pool(name="psCS", bufs=1, space="PSUM") as psCS, \
                     tc.tile_pool(name="psBC", bufs=1, space="PSUM") as psBC, \
                     tc.tile_pool(name="psO", bufs=1, space="PSUM") as psO, \
                     tc.tile_pool(name="p3ps", bufs=2, space="PSUM") as p3ps:
                    pools = dict(psS=psS, psCS=psCS, psBC=psBC, psO=psO,
                                 pt=p2pt, acc=p2acc, p2r=p2r, p2y=p2y)
                    woh_sb = p3w.tile([128, NHC, H], DT.float8e4)
                    wol_sb = p3w.tile([128, NHC, H], DT.float8e4)
                    nc.sync.dma_start(woh_sb[:], woh_r[:])
                    nc.sync.dma_start(wol_sb[:], wol_r[:])

                    ot_state = {}  # tt -> ot tile being filled

                    def oproj_group(g, qj_src):
                        # g in [0, 32): one [128,512] o_proj PSUM group of
                        # strip qj_src; stores batch 4 n-chunks per DMA.
                        tt = 4 * qj_src + g // 8
                        nt = g % 8
                        tsl = slice(tt * 128, (tt + 1) * 128)
                        n0 = nt * 512
                        ps3 = p3ps.tile([128, 512], DT.float32, tag="p3")
                        nmm = 3 * (NHC // 2)
                        i = 0
                        for yt, wt in ((yh, woh_sb), (yl, woh_sb),
                                       (yh, wol_sb)):
                            for j in range(NHC // 2):
                                nc.tensor.matmul(
                                    ps3[:],
                                    yt[:, 2 * j:2 * j + 2, tsl],
                                    wt[:, 2 * j:2 * j + 2, n0:n0 + 512],
                                    start=(i == 0), stop=(i == nmm - 1),
                                    perf_mode=DR)
                                i += 1
                        half, hnt = nt // 4, nt % 4
                        if hnt == 0:
                            ot_new = p3o.tile([128, 4, 512], DT.bfloat16,
                                              tag="ot")
                            ot_state[tt] = ot_new
                        ot = ot_state[tt]
                        with nc.allow_low_precision(reason="bf16 partial out"):
                            nc.vector.tensor_scalar_mul(
                                ot[:, hnt, :], ps3[:], 1.0 / WSCALE)
                        if hnt == 3:
                            # last strip: SP is idle (input stream done) and
                            # drains faster than the Pool SWDGE path
                            eng = nc.sync if qj_src == 3 else nc.gpsimd
                            eng.dma_start(
                                outp[tsl, half * 2048:(half + 1) * 2048]
                                .rearrange("p (h n) -> p h n", h=4),
                                ot[:])

                    nqj = 4 if 2 in phases else 0
                    for qj in range(1, nqj):
                        nki = 4 * qj + 4
                        # o_proj group positions inside the ki loop, 4/head
                        slots = {(si + 1) * nki // 4 - 1: si
                                 for si in range(4)}
                        for h in range(NHC):
                            it = attn_head(qj, h, pools)
                            for ki in it:
                                if 3 in phases and ki in slots:
                                    oproj_group(4 * h + slots[ki], qj - 1)
                        if qj == nqj - 1 and 3 in phases:
                            for g in range(32):
                                oproj_group(g, qj)
    nc.compile()
    return nc


def _get_nc():
    if "nc" not in _CACHE:
        _CACHE["nc"] = _build_nc()
    return _CACHE["nc"]


def _split8(a):
    hi = np.asarray(a, np.float32).astype(F8)
    lo = (np.asarray(a, np.float32) - hi.astype(np.float32)).astype(F8)
    return hi, lo


def _host_prep(hidden_states, position_ids, wq, wk, wv, wo):
    """Build the 8 per-core input maps (all DMA-friendly layouts)."""
    inv_freq = 1.0 / (ROPE_THETA ** (np.arange(0, D, 2, dtype=np.float32) / D))
    p = np.arange(128)[:, None]
    f = np.arange(128)[None, :]
    maskT = np.ascontiguousarray((p <= f).astype(BF16))

    wq = np.asarray(wq, np.float32)
    wk = np.asarray(wk, np.float32)
    wv = np.asarray(wv, np.float32)
    wo = np.asarray(wo, np.float32)

    in_maps = []
    for c in range(8):
        b, g = divmod(c, TP)
        # X^T [H, S] -> [p, ko, strip, 512] -> [strip, p, ko*512]
        xT = np.asarray(hidden_states[b], dtype=np.float32).T
        x4 = xT.reshape(KO, 128, NSTRIP, 512).transpose(2, 1, 0, 3)
        x4 = np.ascontiguousarray(x4).reshape(NSTRIP, 128, KO * 512)
        xh_, xl_ = _split8(x4)
        # Q+K weights [out, H] -> [out=10 slots of 128 dims][p][ko*128]
        wqk = np.concatenate([wq[DQ * g:DQ * (g + 1)],
                              wk[DKV * g:DKV * (g + 1)]], axis=0) * WSCALE
        wqkT = wqk.T.reshape(KO, 128, NPROJ, 128).transpose(2, 1, 0, 3)
        wqkT = np.ascontiguousarray(wqkT).reshape(NPROJ, 128, KO * 128)
        wh_, wl_ = _split8(wqkT)
        # V weights [H, DKV] -> [p, ko*DKV]
        wvT = (wv[DKV * g:DKV * (g + 1)].T * WSCALE)
        wvT = wvT.reshape(KO, 128, DKV).transpose(1, 0, 2)
        wvT = np.ascontiguousarray(wvT).reshape(128, KO * DKV)
        wvh_, wvl_ = _split8(wvT)
        # o_proj weights [DQ, H] -> [p, head*H]
        woT = (wo[:, DQ * g:DQ * (g + 1)].T * WSCALE)  # [DQ, H]
        woTb = woT.reshape(NHC, 128, H).transpose(1, 0, 2)
        woTb = np.ascontiguousarray(woTb).reshape(128, NHC * H)
        woh_, wol_ = _split8(woTb)
        pos = np.asarray(position_ids[b], dtype=np.float32)
        freqs = pos[:, None] * inv_freq[None, :]            # [S, 64]
        cos = np.cos(freqs).T                                # [64, S]
        sin = np.sin(freqs).T
        cosF = np.ascontiguousarray(
            np.concatenate([cos, cos], axis=0).astype(np.float16))
        snF = np.ascontiguousarray(
            np.concatenate([-sin, sin], axis=0).astype(np.float16))
        in_maps.append({
            "xh": xh_, "xl": xl_, "wh": wh_, "wl": wl_,
            "wvh": wvh_, "wvl": wvl_, "woh": woh_, "wol": wol_,
            "cosF": cosF, "snF": snF, "maskT": maskT,
        })
    return in_maps


def _run(in_maps, trace=False):
    from concourse.bass_utils import run_bass_kernel_spmd
    nc = _get_nc()
    return run_bass_kernel_spmd(nc, in_maps, list(range(8)))


def kernel(hidden_states, position_ids, wq, wk, wv, wo):
    hidden_states = np.asarray(hidden_states)
    in_maps = _host_prep(hidden_states, position_ids, wq, wk, wv, wo)
    res = _run(in_maps)
    out = np.zeros((DP, S, H), dtype=np.float32)
    for c in range(8):
        b = c // TP
        out[b] += res.results[c]["outp"].astype(np.float32)
    return out


# revision 57
# speedup vs baseline: 1.0052x; 1.0052x over previous
"""Mistral attention (B=2, S=2048, H=4096, 32 q heads / 8 kv heads, rope) on
8 Trainium2 NeuronCores.

Sharding: DP=2 over batch x TP=4 over head groups. Core c handles batch
c//4 and q-heads 8g..8g+7 (kv heads 2g, 2g+1) where g = c%4. Attention is
fully local per core; each core produces a partial o_proj output
[2048, 4096] (bf16, contribution of its head group) and the host sums the
four TP partials per batch in fp32.

Mixed precision (validated on hw against the fp32 reference, relmax ~3e-3):
  - All projection matmuls (Q/K/V, o_proj) run as fp8-e4m3 DoubleRow
    matmuls (contraction 256/instr at 0.5 cycles/row) with a 3-term
    residual expansion  x_hi*w_hi + x_hi*w_lo + x_lo*w_hi  where
    t_hi = fp8(t), t_lo = fp8(t - t_hi).  X and all weights are split on
    the host; the o_proj input Y is split on-device during softmax
    normalization.  Weights are pre-scaled by 64 so fp8 quantization of
    W*64 stays in the normal range; the 64s cancel via the exp scale, the
    64-valued ones-vector in the denominator reduction, and a 1/64 scale
    on the output store.
  - Attention stays >= bf16 precision: scores fp16 x fp16, P bf16,
    PV bf16 x bf16, softmax denominator accumulated in bf16 (DVE 2x).
    The reciprocal and its PE-broadcast are bf16: denominators reach ~1e7
    so the reciprocal needs bf16's exponent range (fp16 underflows).
  - RoPE in [d, t] layout: 4 DVE ops reading the fp32 projection PSUM
    directly (walrus allows partition-shifted reads only for PSUM inputs)
    with host tables cosF=[cos;cos], snF=[-sin;+sin], fp16 out.

Schedule: PE engines execute in program order, so the program is laid out
to keep PE saturated:
  - Phase 1 streams X strips + per-output weights (all DMAs are
    per-partition contiguous, >=4KB segments, issued on SP in consumption
    order with double-buffered pools).
  - The q-strip-0 attention (ACT-heavy, little PE work) is hoisted into
    phase 1, one head after each strip-2/3 projection job, where the ACT
    engine is otherwise idle.  It uses a dedicated small PSUM pool set.
  - In phase 2, each remaining q-strip's attention interleaves the
    PREVIOUS strip's o_proj PSUM groups inside the ki loop (4 per head),
    giving PE filler while ACT exps drain.  Output stores batch 4
    n-chunks per DMA and issue on the idle GPSIMD queue.
  - Scores/exp tiles on the causal diagonal are restricted to their valid
    q-span; a single [128,128] lower-triangle mask handles the partial
    window.
"""
import sys

if "/opt/trn_rl_repo" not in sys.path:
    sys.path.insert(0, "/opt/trn_rl_repo")

import numpy as np
import ml_dtypes

BF16 = ml_dtypes.bfloat16
F8 = ml_dtypes.float8_e4m3

S = 2048          # sequence length per core
H = 4096          # hidden
D = 128           # head dim
TP = 4            # head-group shards
DP = 2            # batch shards
NHC = 8           # q heads per core
NKVC = 2          # kv heads per core
NPROJ = NHC + NKVC  # 10 projection outputs of 128 dims (8 q heads + 2 kv)
DQ = NHC * D      # 1024 q-proj out dims per core
DKV = NKVC * D    # 256 kv-proj out dims per core
KO = H // 128     # 32 contraction tiles for projections
NT = S // 128     # 16 t tiles
NSTRIP = S // 512  # 4 t strips
SCALE = 1.0 / np.sqrt(D)
ROPE_THETA = 10000.0
WSCALE = 64.0     # host premultiplier on all weights (fp8 range)

_CACHE = {}


def _mybir():
    import concourse.mybir as mybir
    return mybir


def _build_nc(phases=(1, 2, 3), repeats=1, tweaks=()):
    import concourse.mybir as mybir
    import concourse.tile as tile
    from concourse import bacc

    DT = mybir.dt
    DR = mybir.MatmulPerfMode.DoubleRow
    nc = bacc.Bacc(None, target_bir_lowering=False)

    # X strips, per-partition contiguous: [strip][p][ko*512]
    xh = nc.dram_tensor("xh", [NSTRIP, 128, KO * 512], DT.float8e4, kind="ExternalInput")
    xl = nc.dram_tensor("xl", [NSTRIP, 128, KO * 512], DT.float8e4, kind="ExternalInput")
    # Q+K weights, per-output contiguous: [out][p][ko*128]
    wh = nc.dram_tensor("wh", [NPROJ, 128, KO * 128], DT.float8e4, kind="ExternalInput")
    wl = nc.dram_tensor("wl", [NPROJ, 128, KO * 128], DT.float8e4, kind="ExternalInput")
    wvh = nc.dram_tensor("wvh", [128, KO * DKV], DT.float8e4, kind="ExternalInput")
    wvl = nc.dram_tensor("wvl", [128, KO * DKV], DT.float8e4, kind="ExternalInput")
    woh = nc.dram_tensor("woh", [128, NHC * H], DT.float8e4, kind="ExternalInput")
    wol = nc.dram_tensor("wol", [128, NHC * H], DT.float8e4, kind="ExternalInput")
    cosF = nc.dram_tensor("cosF", [D, S], DT.float16, kind="ExternalInput")
    snF = nc.dram_tensor("snF", [D, S], DT.float16, kind="ExternalInput")
    maskT = nc.dram_tensor("maskT", [128, 128], DT.bfloat16, kind="ExternalInput")
    outp = nc.dram_tensor("outp", [S, H], DT.bfloat16, kind="ExternalOutput")

    xh_r = xh.rearrange("s p (ko t) -> s p ko t", ko=KO)      # [4, 128, 32, 512]
    xl_r = xl.rearrange("s p (ko t) -> s p ko t", ko=KO)
    wh_r = wh.rearrange("o p (ko m) -> o p ko m", ko=KO)      # [10, 128, 32, 128]
    wl_r = wl.rearrange("o p (ko m) -> o p ko m", ko=KO)
    wvh_r = wvh.rearrange("p (ko m) -> p ko m", ko=KO)        # [128, 32, 256]
    wvl_r = wvl.rearrange("p (ko m) -> p ko m", ko=KO)
    woh_r = woh.rearrange("p (h n) -> p h n", h=NHC)          # [128, 8, 4096]
    wol_r = wol.rearrange("p (h n) -> p h n", h=NHC)

    EXP = mybir.ActivationFunctionType.Exp

    with tile.TileContext(nc) as tc:
        with tc.tile_pool(name="persist", bufs=1) as persist:
            qT = persist.tile([128, NHC, S], DT.float16)    # [d, head, t]
            kT = persist.tile([128, NKVC, S], DT.float16)   # [d, kv, t]
            vsb = persist.tile([128, NT, DKV], DT.bfloat16)  # [t%128, ttile, dv]
            yh = persist.tile([128, NHC, S], DT.float8e4)   # [d, head, q] hi
            yl = persist.tile([128, NHC, S], DT.float8e4)   # [d, head, q] lo
            mask_sb = persist.tile([128, 128], DT.bfloat16)
            ones = persist.tile([128, 1], DT.bfloat16)
            nc.vector.memset(ones[:], WSCALE)
            ones1 = persist.tile([1, 128], DT.bfloat16)
            nc.vector.memset(ones1[:], 1.0)

            def attn_head(qj, h, P):
                """One head's attention for q-strip qj: scores + exp + mask +
                denominator + PV + normalize + fp8 y split.  P = pool dict."""
                q0 = qj * 512
                nki = 4 * qj + 4
                kv = h // (NHC // NKVC)
                # cs (denominator) and bc (broadcast) have serialized
                # lifetimes: share one PSUM bank (cs = partition-0 row)
                ps_cb = P["psCB"].tile([128, 512], DT.float32, tag="cb")
                ps_cs = ps_cb[0:1, :]
                ps_o = P["psO"].tile([128, 512], DT.float32, tag="o")
                acc = P["acc"].tile([128, 512], DT.bfloat16, tag="acc")
                for ki in range(nki):
                    k0 = ki * 128
                    r = ki - 4 * qj  # >=0 on the causal diagonal
                    v0 = max(r, 0) * 128  # valid q-span start
                    w_ = 512 - v0
                    ps_s = P["psS"].tile([128, 512], DT.float32, tag="s")
                    nc.tensor.matmul(
                        ps_s[:, 0:w_], kT[:, kv, k0:k0 + 128],
                        qT[:, h, q0 + v0:q0 + 512],
                        start=True, stop=True)
                    pt = P["pt"].tile([128, 512], DT.bfloat16, tag="pt")
                    nc.scalar.activation(pt[:, 0:w_], ps_s[:, 0:w_], EXP,
                                         scale=SCALE / (WSCALE * WSCALE))
                    if r >= 0:
                        # triangular window: cols [v0, v0+128)
                        nc.vector.tensor_mul(pt[:, 0:128], pt[:, 0:128],
                                             mask_sb[:])
                    if ki == 0:
                        nc.vector.tensor_copy(acc[:], pt[:])
                    else:
                        nc.vector.tensor_add(acc[:, v0:512], acc[:, v0:512],
                                             pt[:, 0:w_])
                    nc.tensor.matmul(
                        ps_o[:, v0:512],
                        vsb[:, ki, kv * 128:(kv + 1) * 128], pt[:, 0:w_],
                        start=(ki == 0), stop=(ki == nki - 1))
                    yield ki  # interleave point for the caller
                # denominator: 64 * sum_k pt via ones-matmul
                nc.tensor.matmul(ps_cs, ones[:], acc[:], start=True,
                                 stop=True)
                # recip values go down to ~1e-8 (big exp sums): needs bf16's
                # exponent range (fp16 underflows)
                recip = P["p2r"].tile([1, 512], DT.bfloat16, tag="recip")
                with nc.allow_low_precision(reason="bf16 softmax recip"):
                    nc.vector.reciprocal(recip[:], ps_cs)
                # broadcast recip across partitions via K=1 matmul, reusing
                # the cs bank (recip already consumed it)
                ps_bc = ps_cb
                nc.tensor.matmul(ps_bc[:], ones1[:], recip[:], start=True,
                                 stop=True)
                # early PSUM evac in fp32 (ps_o is the UNnormalized sum of
                # exps -- huge; fp16 would overflow), then normalize + split
                yun = P["p2y"].tile([128, 512], DT.float32, tag="yun")
                nc.vector.tensor_copy(yun[:], ps_o[:])
                ytmp = P["p2y"].tile([128, 512], DT.float16, tag="ytmp")
                nc.vector.tensor_mul(ytmp[:], yun[:], ps_bc[:])
                nc.scalar.copy(yh[:, h, q0:q0 + 512], ytmp[:])
                with nc.allow_low_precision(reason="fp8 residual"):
                    nc.vector.tensor_sub(yl[:, h, q0:q0 + 512], ytmp[:],
                                         yh[:, h, q0:q0 + 512])

            for _rep in range(repeats):
                # ---------------- Phase 1: projections + rope ----------------
                with tc.tile_pool(name="p1c", bufs=1) as p1c, \
                     tc.tile_pool(name="p1x", bufs=2) as p1x, \
                     tc.tile_pool(name="p1w", bufs=2) as p1w, \
                     tc.tile_pool(name="p1t", bufs=1) as p1t, \
                     tc.tile_pool(name="q0pt", bufs=8) as q0pt, \
                     tc.tile_pool(name="q0acc", bufs=2) as q0acc, \
                     tc.tile_pool(name="q0r", bufs=2) as q0r, \
                     tc.tile_pool(name="q0y", bufs=2) as q0y, \
                     tc.tile_pool(name="p1ps", bufs=2, space="PSUM") as p1ps, \
                     tc.tile_pool(name="p1pv", bufs=1, space="PSUM") as p1pv, \
                     tc.tile_pool(name="q0S", bufs=3, space="PSUM") as q0S, \
                     tc.tile_pool(name="q0CB", bufs=1, space="PSUM") as q0CB, \
                     tc.tile_pool(name="q0O", bufs=1, space="PSUM") as q0O:
                    q0pools = dict(psS=q0S, psCB=q0CB, psO=q0O,
                                   pt=q0pt, acc=q0acc, p2r=q0r, p2y=q0y)
                    cos_sb = p1c.tile([D, S], DT.float16)
                    sn_sb = p1c.tile([D, S], DT.float16)
                    wvh_sb = p1c.tile([128, KO, DKV], DT.float8e4)
                    wvl_sb = p1c.tile([128, KO, DKV], DT.float8e4)

                    def rope(dst, ps, t0):
                        # dst [128, 512] fp16 (partition = d within head);
                        # rotate_half across the partition halves:
                        #   dst = ps*cosF + swap64(ps)*snF
                        # with cosF = [cos;cos], snF = [-sin;+sin].  The
                        # partition-shifted reads keep ps in PSUM (walrus
                        # forbids mismatched base partitions for SBUF+SBUF
                        # TensorTensor inputs, but PSUM+SBUF is fine).
                        u = p1t.tile([128, 512], DT.float16, tag="ru")
                        c = p1t.tile([128, 512], DT.float16, tag="rc")
                        nc.vector.tensor_mul(u[0:64, :], ps[64:128, :],
                                             sn_sb[0:64, t0:t0 + 512])
                        nc.vector.tensor_mul(u[64:128, :], ps[0:64, :],
                                             sn_sb[64:128, t0:t0 + 512])
                        nc.vector.tensor_mul(c[:], ps[:],
                                             cos_sb[:, t0:t0 + 512])
                        nc.vector.tensor_add(dst[:], c[:], u[:])

                    def load_w(h):
                        wth = p1w.tile([128, KO, 128], DT.float8e4, tag="wh")
                        wtl = p1w.tile([128, KO, 128], DT.float8e4, tag="wl")
                        nc.sync.dma_start(wth[:], wh_r[h])
                        nc.sync.dma_start(wtl[:], wl_r[h])
                        return wth, wtl

                    def load_x(ts):
                        xsh = p1x.tile([128, KO, 512], DT.float8e4, tag="xsh")
                        xsl = p1x.tile([128, KO, 512], DT.float8e4, tag="xsl")
                        return xsh, xsl

                    def x_chunk(dst, src_r, ts, c, nchunks=4):
                        ko_per = KO // nchunks
                        k0 = c * ko_per
                        nc.sync.dma_start(
                            dst[:, k0:k0 + ko_per, :],
                            src_r[ts, :, k0:k0 + ko_per, :])

                    def proj_group(ps, wth, wtl, xsh, xsl):
                        # half-ktile blocks: early matmuls only need the
                        # first half of each X tensor (startup latency)
                        nmm = 3 * (KO // 2)
                        i = 0
                        for jblk in range(2):
                            for wt, xt in ((wth, xsh), (wtl, xsh), (wth, xsl)):
                                for j in range(jblk * (KO // 4),
                                               (jblk + 1) * (KO // 4)):
                                    nc.tensor.matmul(
                                        ps[:], wt[:, 2 * j:2 * j + 2, :],
                                        xt[:, 2 * j:2 * j + 2, :],
                                        start=(i == 0), stop=(i == nmm - 1),
                                        perf_mode=DR)
                                    i += 1

                    strips = list(range(NSTRIP if 1 in phases else 0))
                    xs_cur = None
                    q0_queue = []  # pending hoisted (qj, h) attention heads
                    q0_iter = [None]  # suspended attn_head generator

                    def pump_q0():
                        if q0_iter[0] is None and q0_queue:
                            qj_, h_ = q0_queue.pop(0)
                            q0_iter[0] = attn_head(qj_, h_, q0pools)
                        if q0_iter[0] is not None:
                            try:
                                next(q0_iter[0])
                            except StopIteration:
                                q0_iter[0] = None

                    for ts in strips:
                        t0 = ts * 512
                        if ts == 0:
                            # startup order follows consumption deadlines of
                            # the half-ktile-blocked matmul order below
                            wth0 = p1w.tile([128, KO, 128], DT.float8e4, tag="wh")
                            nc.sync.dma_start(wth0[:], wh_r[0])
                            xs_cur = load_x(0)
                            x_chunk(xs_cur[0], xh_r, 0, 0)
                            wtl0 = p1w.tile([128, KO, 128], DT.float8e4, tag="wl")
                            nc.sync.dma_start(wtl0[:], wl_r[0])
                            w_cur = (wth0, wtl0)
                            x_chunk(xs_cur[0], xh_r, 0, 1)
                            x_chunk(xs_cur[1], xl_r, 0, 0)
                            x_chunk(xs_cur[1], xl_r, 0, 1)
                            nc.sync.dma_start(cos_sb[:], cosF[:])
                            nc.sync.dma_start(sn_sb[:], snF[:])
                            x_chunk(xs_cur[0], xh_r, 0, 2)
                            x_chunk(xs_cur[0], xh_r, 0, 3)
                            x_chunk(xs_cur[1], xl_r, 0, 2)
                            x_chunk(xs_cur[1], xl_r, 0, 3)
                            nc.sync.dma_start(mask_sb[:], maskT[:])
                        xsh, xsl = xs_cur
                        xs_nxt = load_x(ts + 1) if ts + 1 in strips else None
                        # Q^T heads and K^T kv-heads ([d, t] layout)
                        for h in range(NPROJ):
                            wth, wtl = w_cur
                            # prefetch next output's weights (and, spread over
                            # the loop, this/next strip's X chunks + consts)
                            w_nxt = None
                            if h + 1 < NPROJ:
                                w_nxt = load_w(h + 1)
                            elif ts + 1 in strips:
                                w_nxt = load_w(0)
                            if ts == 0:
                                if h == 6:
                                    nc.sync.dma_start(wvh_sb[:], wvh_r[:])
                                elif h == 7:
                                    nc.sync.dma_start(wvl_sb[:], wvl_r[:])
                            if 2 <= h < 6 and xs_nxt is not None:
                                x_chunk(xs_nxt[0], xh_r, ts + 1, h - 2)
                            elif 6 <= h and xs_nxt is not None:
                                x_chunk(xs_nxt[1], xl_r, ts + 1, h - 6)
                            ps = p1ps.tile([128, 512], DT.float32, tag="proj")
                            proj_group(ps, wth, wtl, xsh, xsl)
                            if h < NHC:
                                rope(qT[:, h, t0:t0 + 512], ps, t0)
                            else:
                                rope(kT[:, h - NHC, t0:t0 + 512], ps, t0)
                            w_cur = w_nxt
                            # hoisted q-strip-0 attention, paced so the heads
                            # stretch into the V region (fills V-evac gaps)
                            if 2 in phases and 1 in phases:
                                if ts == 2 and h == 0:
                                    q0_queue.extend((0, hh) for hh in range(NHC))
                                for _ in range(3):
                                    pump_q0()
                        # V natural layout [t, dv]
                        if 2 in phases and 1 in phases and ts == 3:
                            # fill strip-3's V-evac gaps with q-strip-1 head 0
                            q0_queue.append((1, 0))
                        for mt in range(4):
                            tsl = slice(mt * 128, (mt + 1) * 128)
                            psv = p1pv.tile([128, DKV], DT.float32, tag="pv")
                            nmm = 3 * (KO // 2)
                            i = 0
                            for xt, wt in ((xsh, wvh_sb), (xsh, wvl_sb), (xsl, wvh_sb)):
                                for j in range(KO // 2):
                                    nc.tensor.matmul(
                                        psv[:], xt[:, 2 * j:2 * j + 2, tsl],
                                        wt[:, 2 * j:2 * j + 2, :],
                                        start=(i == 0), stop=(i == nmm - 1),
                                        perf_mode=DR)
                                    i += 1
                            nc.scalar.copy(vsb[:, ts * 4 + mt, :], psv[:])
                            for _ in range(3):
                                pump_q0()
                        xs_cur = xs_nxt
                    # drain any remaining q-strip-0 attention
                    while q0_iter[0] is not None or q0_queue:
                        pump_q0()

                # ------- Phases 2+3 interleaved, strip-pipelined -------
                with tc.tile_pool(name="p2pt", bufs=12) as p2pt, \
                     tc.tile_pool(name="p2r", bufs=2) as p2r, \
                     tc.tile_pool(name="p2acc", bufs=2) as p2acc, \
                     tc.tile_pool(name="p2y", bufs=3) as p2y, \
                     tc.tile_pool(name="p3w", bufs=1) as p3w, \
                     tc.tile_pool(name="p3o", bufs=3) as p3o, \
                     tc.tile_pool(name="psS", bufs=4, space="PSUM") as psS, \
                     tc.tile_pool(name="psCB", bufs=1, space="PSUM") as psCB, \
                     tc.tile_pool(name="psO", bufs=1, space="PSUM") as psO, \
                     tc.tile_pool(name="p3ps", bufs=2, space="PSUM") as p3ps:
                    pools = dict(psS=psS, psCB=psCB, psO=psO,
                                 pt=p2pt, acc=p2acc, p2r=p2r, p2y=p2y)
                    woh_sb = p3w.tile([128, NHC, H], DT.float8e4)
                    wol_sb = p3w.tile([128, NHC, H], DT.float8e4)
                    nc.sync.dma_start(woh_sb[:], woh_r[:])
                    nc.sync.dma_start(wol_sb[:], wol_r[:])

                    ot_state = {}  # tt -> ot tile being filled

                    def oproj_group(g, qj_src):
                        # g in [0, 32): one [128,512] o_proj PSUM group of
                        # strip qj_src; stores batch 4 n-chunks per DMA.
                        tt = 4 * qj_src + g // 8
                        nt = g % 8
                        tsl = slice(tt * 128, (tt + 1) * 128)
                        n0 = nt * 512
                        ps3 = p3ps.tile([128, 512], DT.float32, tag="p3")
                        nmm = 3 * (NHC // 2)
                        i = 0
                        for yt, wt in ((yh, woh_sb), (yl, woh_sb),
                                       (yh, wol_sb)):
                            for j in range(NHC // 2):
                                nc.tensor.matmul(
                                    ps3[:],
                                    yt[:, 2 * j:2 * j + 2, tsl],
                                    wt[:, 2 * j:2 * j + 2, n0:n0 + 512],
                                    start=(i == 0), stop=(i == nmm - 1),
                                    perf_mode=DR)
                                i += 1
                        # batch 4 n-chunks per store DMA; the final strip uses
                        # quarter stores on the idle SP queue (shorter drain)
                        per = 2 if qj_src == 3 else 4
                        half, hnt = nt // per, nt % per
                        if hnt == 0:
                            ot_new = p3o.tile([128, per, 512], DT.bfloat16,
                                              tag=f"ot{per}")
                            ot_state[tt] = ot_new
                        ot = ot_state[tt]
                        with nc.allow_low_precision(reason="bf16 partial out"):
                            nc.vector.tensor_scalar_mul(
                                ot[:, hnt, :], ps3[:], 1.0 / WSCALE)
                        if hnt == per - 1:
                            eng = nc.sync if qj_src == 3 else nc.gpsimd
                            eng.dma_start(
                                outp[tsl, half * 512 * per:(half + 1) * 512 * per]
                                .rearrange("p (h n) -> p h n", h=per),
                                ot[:])

                    nqj = 4 if 2 in phases else 0
                    for qj in range(1, nqj):
                        nki = 4 * qj + 4
                        # o_proj group positions inside the ki loop, 4/head
                        slots = {(si + 1) * nki // 4 - 1: si
                                 for si in range(4)}
                        # qj=1 head 0 was hoisted into phase 1; its 4 o_proj
                        # groups run as a prefix block instead
                        h0 = 1 if qj == 1 else 0
                        if 3 in phases and qj == 1 and 1 in phases:
                            for g in range(4):
                                oproj_group(g, 0)
                        for h in range(h0, NHC):
                            it = attn_head(qj, h, pools)
                            for ki in it:
                                if 3 in phases and ki in slots:
                                    oproj_group(4 * h + slots[ki], qj - 1)
                        if qj == nqj - 1 and 3 in phases:
                            for g in range(32):
                                oproj_group(g, qj)
    nc.compile()
    return nc


def _get_nc():
    if "nc" not in _CACHE:
        _CACHE["nc"] = _build_nc()
    return _CACHE["nc"]


def _split8(a):
    hi = np.asarray(a, np.float32).astype(F8)
    lo = (np.asarray(a, np.float32) - hi.astype(np.float32)).astype(F8)
    return hi, lo


def _host_prep(hidden_states, position_ids, wq, wk, wv, wo):
    """Build the 8 per-core input maps (all DMA-friendly layouts)."""
    inv_freq = 1.0 / (ROPE_THETA ** (np.arange(0, D, 2, dtype=np.float32) / D))
    p = np.arange(128)[:, None]
    f = np.arange(128)[None, :]
    maskT = np.ascontiguousarray((p <= f).astype(BF16))

    wq = np.asarray(wq, np.float32)
    wk = np.asarray(wk, np.float32)
    wv = np.asarray(wv, np.float32)
    wo = np.asarray(wo, np.float32)

    in_maps = []
    for c in range(8):
        b, g = divmod(c, TP)
        # X^T [H, S] -> [p, ko, strip, 512] -> [strip, p, ko*512]
        xT = np.asarray(hidden_states[b], dtype=np.float32).T
        x4 = xT.reshape(KO, 128, NSTRIP, 512).transpose(2, 1, 0, 3)
        x4 = np.ascontiguousarray(x4).reshape(NSTRIP, 128, KO * 512)
        xh_, xl_ = _split8(x4)
        # Q+K weights [out, H] -> [out=10 slots of 128 dims][p][ko*128]
        wqk = np.concatenate([wq[DQ * g:DQ * (g + 1)],
                              wk[DKV * g:DKV * (g + 1)]], axis=0) * WSCALE
        wqkT = wqk.T.reshape(KO, 128, NPROJ, 128).transpose(2, 1, 0, 3)
        wqkT = np.ascontiguousarray(wqkT).reshape(NPROJ, 128, KO * 128)
        wh_, wl_ = _split8(wqkT)
        # V weights [H, DKV] -> [p, ko*DKV]
        wvT = (wv[DKV * g:DKV * (g + 1)].T * WSCALE)
        wvT = wvT.reshape(KO, 128, DKV).transpose(1, 0, 2)
        wvT = np.ascontiguousarray(wvT).reshape(128, KO * DKV)
        wvh_, wvl_ = _split8(wvT)
        # o_proj weights [DQ, H] -> [p, head*H]
        woT = (wo[:, DQ * g:DQ * (g + 1)].T * WSCALE)  # [DQ, H]
        woTb = woT.reshape(NHC, 128, H).transpose(1, 0, 2)
        woTb = np.ascontiguousarray(woTb).reshape(128, NHC * H)
        woh_, wol_ = _split8(woTb)
        pos = np.asarray(position_ids[b], dtype=np.float32)
        freqs = pos[:, None] * inv_freq[None, :]            # [S, 64]
        cos = np.cos(freqs).T                                # [64, S]
        sin = np.sin(freqs).T
        cosF = np.ascontiguousarray(
            np.concatenate([cos, cos], axis=0).astype(np.float16))
        snF = np.ascontiguousarray(
            np.concatenate([-sin, sin], axis=0).astype(np.float16))
        in_maps.append({
            "xh": xh_, "xl": xl_, "wh": wh_, "wl": wl_,
            "wvh": wvh_, "wvl": wvl_, "woh": woh_, "wol": wol_,
            "cosF": cosF, "snF": snF, "maskT": maskT,
        })
    return in_maps


def _run(in_maps, trace=False):
    from concourse.bass_utils import run_bass_kernel_spmd
    nc = _get_nc()
    return run_bass_kernel_spmd(nc, in_maps, list(range(8)))


def kernel(hidden_states, position_ids, wq, wk, wv, wo):
    hidden_states = np.asarray(hidden_states)
    in_maps = _host_prep(hidden_states, position_ids, wq, wk, wv, wo)
    res = _run(in_maps)
    out = np.zeros((DP, S, H), dtype=np.float32)
    for c in range(8):
        b = c // TP
        out[b] += res.results[c]["outp"].astype(np.float32)
    return out


# revision 58
# speedup vs baseline: 1.0064x; 1.0012x over previous
"""Mistral attention (B=2, S=2048, H=4096, 32 q heads / 8 kv heads, rope) on
8 Trainium2 NeuronCores.

Sharding: DP=2 over batch x TP=4 over head groups. Core c handles batch
c//4 and q-heads 8g..8g+7 (kv heads 2g, 2g+1) where g = c%4. Attention is
fully local per core; each core produces a partial o_proj output
[2048, 4096] (bf16, contribution of its head group) and the host sums the
four TP partials per batch in fp32.

Mixed precision (validated on hw against the fp32 reference, relmax ~3e-3):
  - All projection matmuls (Q/K/V, o_proj) run as fp8-e4m3 DoubleRow
    matmuls (contraction 256/instr at 0.5 cycles/row) with a 3-term
    residual expansion  x_hi*w_hi + x_hi*w_lo + x_lo*w_hi  where
    t_hi = fp8(t), t_lo = fp8(t - t_hi).  X and all weights are split on
    the host; the o_proj input Y is split on-device during softmax
    normalization.  Weights are pre-scaled by 64 so fp8 quantization of
    W*64 stays in the normal range; the 64s cancel via the exp scale, the
    64-valued ones-vector in the denominator reduction, and a 1/64 scale
    on the output store.
  - Attention stays >= bf16 precision: scores fp16 x fp16, P bf16,
    PV bf16 x bf16, softmax denominator accumulated in bf16 (DVE 2x).
    The reciprocal and its PE-broadcast are bf16: denominators reach ~1e7
    so the reciprocal needs bf16's exponent range (fp16 underflows).
  - RoPE in [d, t] layout: 4 DVE ops reading the fp32 projection PSUM
    directly (walrus allows partition-shifted reads only for PSUM inputs)
    with host tables cosF=[cos;cos], snF=[-sin;+sin], fp16 out.

Schedule: PE engines execute in program order, so the program is laid out
to keep PE saturated:
  - Phase 1 streams X strips + per-output weights (all DMAs are
    per-partition contiguous, >=4KB segments, issued on SP in consumption
    order with double-buffered pools).
  - The q-strip-0 attention (ACT-heavy, little PE work) is hoisted into
    phase 1, one head after each strip-2/3 projection job, where the ACT
    engine is otherwise idle.  It uses a dedicated small PSUM pool set.
  - In phase 2, each remaining q-strip's attention interleaves the
    PREVIOUS strip's o_proj PSUM groups inside the ki loop (4 per head),
    giving PE filler while ACT exps drain.  Output stores batch 4
    n-chunks per DMA and issue on the idle GPSIMD queue.
  - Scores/exp tiles on the causal diagonal are restricted to their valid
    q-span; a single [128,128] lower-triangle mask handles the partial
    window.
"""
import sys

if "/opt/trn_rl_repo" not in sys.path:
    sys.path.insert(0, "/opt/trn_rl_repo")

import numpy as np
import ml_dtypes

BF16 = ml_dtypes.bfloat16
F8 = ml_dtypes.float8_e4m3

S = 2048          # sequence length per core
H = 4096          # hidden
D = 128           # head dim
TP = 4            # head-group shards
DP = 2            # batch shards
NHC = 8           # q heads per core
NKVC = 2          # kv heads per core
NPROJ = NHC + NKVC  # 10 projection outputs of 128 dims (8 q heads + 2 kv)
DQ = NHC * D      # 1024 q-proj out dims per core
DKV = NKVC * D    # 256 kv-proj out dims per core
KO = H // 128     # 32 contraction tiles for projections
NT = S // 128     # 16 t tiles
NSTRIP = S // 512  # 4 t strips
SCALE = 1.0 / np.sqrt(D)
ROPE_THETA = 10000.0
WSCALE = 64.0     # host premultiplier on all weights (fp8 range)

_CACHE = {}


def _mybir():
    import concourse.mybir as mybir
    return mybir


def _build_nc(phases=(1, 2, 3), repeats=1, tweaks=()):
    import concourse.mybir as mybir
    import concourse.tile as tile
    from concourse import bacc

    DT = mybir.dt
    DR = mybir.MatmulPerfMode.DoubleRow
    nc = bacc.Bacc(None, target_bir_lowering=False)

    # X strips, per-partition contiguous: [strip][p][ko*512]
    xh = nc.dram_tensor("xh", [NSTRIP, 128, KO * 512], DT.float8e4, kind="ExternalInput")
    xl = nc.dram_tensor("xl", [NSTRIP, 128, KO * 512], DT.float8e4, kind="ExternalInput")
    # Q+K weights, per-output contiguous: [out][p][ko*128]
    wh = nc.dram_tensor("wh", [NPROJ, 128, KO * 128], DT.float8e4, kind="ExternalInput")
    wl = nc.dram_tensor("wl", [NPROJ, 128, KO * 128], DT.float8e4, kind="ExternalInput")
    wvh = nc.dram_tensor("wvh", [128, KO * DKV], DT.float8e4, kind="ExternalInput")
    wvl = nc.dram_tensor("wvl", [128, KO * DKV], DT.float8e4, kind="ExternalInput")
    woh = nc.dram_tensor("woh", [128, NHC * H], DT.float8e4, kind="ExternalInput")
    wol = nc.dram_tensor("wol", [128, NHC * H], DT.float8e4, kind="ExternalInput")
    cosF = nc.dram_tensor("cosF", [D, S], DT.float16, kind="ExternalInput")
    snF = nc.dram_tensor("snF", [D, S], DT.float16, kind="ExternalInput")
    maskT = nc.dram_tensor("maskT", [128, 128], DT.bfloat16, kind="ExternalInput")
    outp = nc.dram_tensor("outp", [S, H], DT.bfloat16, kind="ExternalOutput")

    xh_r = xh.rearrange("s p (ko t) -> s p ko t", ko=KO)      # [4, 128, 32, 512]
    xl_r = xl.rearrange("s p (ko t) -> s p ko t", ko=KO)
    wh_r = wh.rearrange("o p (ko m) -> o p ko m", ko=KO)      # [10, 128, 32, 128]
    wl_r = wl.rearrange("o p (ko m) -> o p ko m", ko=KO)
    wvh_r = wvh.rearrange("p (ko m) -> p ko m", ko=KO)        # [128, 32, 256]
    wvl_r = wvl.rearrange("p (ko m) -> p ko m", ko=KO)
    woh_r = woh.rearrange("p (h n) -> p h n", h=NHC)          # [128, 8, 4096]
    wol_r = wol.rearrange("p (h n) -> p h n", h=NHC)

    EXP = mybir.ActivationFunctionType.Exp

    with tile.TileContext(nc) as tc:
        with tc.tile_pool(name="persist", bufs=1) as persist:
            qT = persist.tile([128, NHC, S], DT.float16)    # [d, head, t]
            kT = persist.tile([128, NKVC, S], DT.float16)   # [d, kv, t]
            vsb = persist.tile([128, NT, DKV], DT.bfloat16)  # [t%128, ttile, dv]
            yh = persist.tile([128, NHC, S], DT.float8e4)   # [d, head, q] hi
            yl = persist.tile([128, NHC, S], DT.float8e4)   # [d, head, q] lo
            mask_sb = persist.tile([128, 128], DT.bfloat16)
            ones = persist.tile([128, 1], DT.bfloat16)
            nc.vector.memset(ones[:], WSCALE)
            ones1 = persist.tile([1, 128], DT.bfloat16)
            nc.vector.memset(ones1[:], 1.0)

            def attn_head(qj, h, P):
                """One head's attention for q-strip qj: scores + exp + mask +
                denominator + PV + normalize + fp8 y split.  P = pool dict."""
                q0 = qj * 512
                nki = 4 * qj + 4
                kv = h // (NHC // NKVC)
                # cs (denominator) and bc (broadcast) have serialized
                # lifetimes: share one PSUM bank (cs = partition-0 row)
                ps_cb = P["psCB"].tile([128, 512], DT.float32, tag="cb")
                ps_cs = ps_cb[0:1, :]
                ps_o = P["psO"].tile([128, 512], DT.float32, tag="o")
                acc = P["acc"].tile([128, 512], DT.bfloat16, tag="acc")
                for ki in range(nki):
                    k0 = ki * 128
                    r = ki - 4 * qj  # >=0 on the causal diagonal
                    v0 = max(r, 0) * 128  # valid q-span start
                    w_ = 512 - v0
                    ps_s = P["psS"].tile([128, 512], DT.float32, tag="s")
                    nc.tensor.matmul(
                        ps_s[:, 0:w_], kT[:, kv, k0:k0 + 128],
                        qT[:, h, q0 + v0:q0 + 512],
                        start=True, stop=True)
                    pt = P["pt"].tile([128, 512], DT.bfloat16, tag="pt")
                    nc.scalar.activation(pt[:, 0:w_], ps_s[:, 0:w_], EXP,
                                         scale=SCALE / (WSCALE * WSCALE))
                    if r >= 0:
                        # triangular window: cols [v0, v0+128)
                        nc.vector.tensor_mul(pt[:, 0:128], pt[:, 0:128],
                                             mask_sb[:])
                    if ki == 0:
                        nc.vector.tensor_copy(acc[:], pt[:])
                    else:
                        nc.vector.tensor_add(acc[:, v0:512], acc[:, v0:512],
                                             pt[:, 0:w_])
                    nc.tensor.matmul(
                        ps_o[:, v0:512],
                        vsb[:, ki, kv * 128:(kv + 1) * 128], pt[:, 0:w_],
                        start=(ki == 0), stop=(ki == nki - 1))
                    yield ki  # interleave point for the caller
                # denominator: 64 * sum_k pt via ones-matmul
                nc.tensor.matmul(ps_cs, ones[:], acc[:], start=True,
                                 stop=True)
                # recip values go down to ~1e-8 (big exp sums): needs bf16's
                # exponent range (fp16 underflows)
                recip = P["p2r"].tile([1, 512], DT.bfloat16, tag="recip")
                with nc.allow_low_precision(reason="bf16 softmax recip"):
                    nc.vector.reciprocal(recip[:], ps_cs)
                # broadcast recip across partitions via K=1 matmul, reusing
                # the cs bank (recip already consumed it)
                ps_bc = ps_cb
                nc.tensor.matmul(ps_bc[:], ones1[:], recip[:], start=True,
                                 stop=True)
                # early PSUM evac in fp32 (ps_o is the UNnormalized sum of
                # exps -- huge; fp16 would overflow), then normalize + split
                yun = P["p2y"].tile([128, 512], DT.float32, tag="yun")
                nc.vector.tensor_copy(yun[:], ps_o[:])
                ytmp = P["p2y"].tile([128, 512], DT.float16, tag="ytmp")
                nc.vector.tensor_mul(ytmp[:], yun[:], ps_bc[:])
                nc.scalar.copy(yh[:, h, q0:q0 + 512], ytmp[:])
                with nc.allow_low_precision(reason="fp8 residual"):
                    nc.vector.tensor_sub(yl[:, h, q0:q0 + 512], ytmp[:],
                                         yh[:, h, q0:q0 + 512])

            for _rep in range(repeats):
                # ---------------- Phase 1: projections + rope ----------------
                with tc.tile_pool(name="p1c", bufs=1) as p1c, \
                     tc.tile_pool(name="p1x", bufs=2) as p1x, \
                     tc.tile_pool(name="p1w", bufs=2) as p1w, \
                     tc.tile_pool(name="p1t", bufs=1) as p1t, \
                     tc.tile_pool(name="q0pt", bufs=8) as q0pt, \
                     tc.tile_pool(name="q0acc", bufs=2) as q0acc, \
                     tc.tile_pool(name="q0r", bufs=2) as q0r, \
                     tc.tile_pool(name="q0y", bufs=2) as q0y, \
                     tc.tile_pool(name="p1ps", bufs=2, space="PSUM") as p1ps, \
                     tc.tile_pool(name="p1pv", bufs=1, space="PSUM") as p1pv, \
                     tc.tile_pool(name="q0S", bufs=3, space="PSUM") as q0S, \
                     tc.tile_pool(name="q0CB", bufs=1, space="PSUM") as q0CB, \
                     tc.tile_pool(name="q0O", bufs=1, space="PSUM") as q0O:
                    q0pools = dict(psS=q0S, psCB=q0CB, psO=q0O,
                                   pt=q0pt, acc=q0acc, p2r=q0r, p2y=q0y)
                    cos_sb = p1c.tile([D, S], DT.float16)
                    sn_sb = p1c.tile([D, S], DT.float16)
                    wvh_sb = p1c.tile([128, KO, DKV], DT.float8e4)
                    wvl_sb = p1c.tile([128, KO, DKV], DT.float8e4)

                    def rope(dst, ps, t0):
                        # dst [128, 512] fp16 (partition = d within head);
                        # rotate_half across the partition halves:
                        #   dst = ps*cosF + swap64(ps)*snF
                        # with cosF = [cos;cos], snF = [-sin;+sin].  The
                        # partition-shifted reads keep ps in PSUM (walrus
                        # forbids mismatched base partitions for SBUF+SBUF
                        # TensorTensor inputs, but PSUM+SBUF is fine).
                        u = p1t.tile([128, 512], DT.float16, tag="ru")
                        c = p1t.tile([128, 512], DT.float16, tag="rc")
                        nc.vector.tensor_mul(u[0:64, :], ps[64:128, :],
                                             sn_sb[0:64, t0:t0 + 512])
                        nc.vector.tensor_mul(u[64:128, :], ps[0:64, :],
                                             sn_sb[64:128, t0:t0 + 512])
                        nc.vector.tensor_mul(c[:], ps[:],
                                             cos_sb[:, t0:t0 + 512])
                        nc.vector.tensor_add(dst[:], c[:], u[:])

                    def load_w(h):
                        wth = p1w.tile([128, KO, 128], DT.float8e4, tag="wh")
                        wtl = p1w.tile([128, KO, 128], DT.float8e4, tag="wl")
                        nc.sync.dma_start(wth[:], wh_r[h])
                        nc.sync.dma_start(wtl[:], wl_r[h])
                        return wth, wtl

                    def load_x(ts):
                        xsh = p1x.tile([128, KO, 512], DT.float8e4, tag="xsh")
                        xsl = p1x.tile([128, KO, 512], DT.float8e4, tag="xsl")
                        return xsh, xsl

                    def x_chunk(dst, src_r, ts, c, nchunks=4):
                        ko_per = KO // nchunks
                        k0 = c * ko_per
                        nc.sync.dma_start(
                            dst[:, k0:k0 + ko_per, :],
                            src_r[ts, :, k0:k0 + ko_per, :])

                    def proj_group(ps, wth, wtl, xsh, xsl):
                        # half-ktile blocks: early matmuls only need the
                        # first half of each X tensor (startup latency)
                        nmm = 3 * (KO // 2)
                        i = 0
                        for jblk in range(2):
                            for wt, xt in ((wth, xsh), (wtl, xsh), (wth, xsl)):
                                for j in range(jblk * (KO // 4),
                                               (jblk + 1) * (KO // 4)):
                                    nc.tensor.matmul(
                                        ps[:], wt[:, 2 * j:2 * j + 2, :],
                                        xt[:, 2 * j:2 * j + 2, :],
                                        start=(i == 0), stop=(i == nmm - 1),
                                        perf_mode=DR)
                                    i += 1

                    strips = list(range(NSTRIP if 1 in phases else 0))
                    xs_cur = None
                    q0_queue = []  # pending hoisted (qj, h) attention heads
                    q0_iter = [None]  # suspended attn_head generator

                    def pump_q0():
                        if q0_iter[0] is None and q0_queue:
                            qj_, h_ = q0_queue.pop(0)
                            q0_iter[0] = attn_head(qj_, h_, q0pools)
                        if q0_iter[0] is not None:
                            try:
                                next(q0_iter[0])
                            except StopIteration:
                                q0_iter[0] = None

                    for ts in strips:
                        t0 = ts * 512
                        if ts == 0:
                            # startup order follows consumption deadlines of
                            # the half-ktile-blocked matmul order below
                            wth0 = p1w.tile([128, KO, 128], DT.float8e4, tag="wh")
                            nc.sync.dma_start(wth0[:], wh_r[0])
                            xs_cur = load_x(0)
                            x_chunk(xs_cur[0], xh_r, 0, 0)
                            wtl0 = p1w.tile([128, KO, 128], DT.float8e4, tag="wl")
                            nc.sync.dma_start(wtl0[:], wl_r[0])
                            w_cur = (wth0, wtl0)
                            x_chunk(xs_cur[0], xh_r, 0, 1)
                            x_chunk(xs_cur[1], xl_r, 0, 0)
                            x_chunk(xs_cur[1], xl_r, 0, 1)
                            nc.sync.dma_start(cos_sb[:], cosF[:])
                            nc.sync.dma_start(sn_sb[:], snF[:])
                            x_chunk(xs_cur[0], xh_r, 0, 2)
                            x_chunk(xs_cur[0], xh_r, 0, 3)
                            x_chunk(xs_cur[1], xl_r, 0, 2)
                            x_chunk(xs_cur[1], xl_r, 0, 3)
                            nc.sync.dma_start(mask_sb[:], maskT[:])
                        xsh, xsl = xs_cur
                        xs_nxt = load_x(ts + 1) if ts + 1 in strips else None
                        # Q^T heads and K^T kv-heads ([d, t] layout)
                        for h in range(NPROJ):
                            wth, wtl = w_cur
                            # prefetch next output's weights (and, spread over
                            # the loop, this/next strip's X chunks + consts)
                            w_nxt = None
                            if h + 1 < NPROJ:
                                w_nxt = load_w(h + 1)
                            elif ts + 1 in strips:
                                w_nxt = load_w(0)
                            if ts == 0:
                                if h == 6:
                                    nc.sync.dma_start(wvh_sb[:], wvh_r[:])
                                elif h == 7:
                                    nc.sync.dma_start(wvl_sb[:], wvl_r[:])
                            if 2 <= h < 6 and xs_nxt is not None:
                                x_chunk(xs_nxt[0], xh_r, ts + 1, h - 2)
                            elif 6 <= h and xs_nxt is not None:
                                x_chunk(xs_nxt[1], xl_r, ts + 1, h - 6)
                            ps = p1ps.tile([128, 512], DT.float32, tag="proj")
                            proj_group(ps, wth, wtl, xsh, xsl)
                            if h < NHC:
                                rope(qT[:, h, t0:t0 + 512], ps, t0)
                            else:
                                rope(kT[:, h - NHC, t0:t0 + 512], ps, t0)
                            w_cur = w_nxt
                            # hoisted q-strip-0 attention, paced so the heads
                            # stretch into the V region (fills V-evac gaps)
                            if 2 in phases and 1 in phases:
                                if ts == 2 and h == 0:
                                    q0_queue.extend((0, hh) for hh in range(NHC))
                                for _ in range(3):
                                    pump_q0()
                        # V natural layout [t, dv]
                        if 2 in phases and 1 in phases and ts == 3:
                            # fill strip-3's V-evac gaps with q-strip-1 head 0
                            q0_queue.append((1, 0))
                        for mt in range(4):
                            tsl = slice(mt * 128, (mt + 1) * 128)
                            psv = p1pv.tile([128, DKV], DT.float32, tag="pv")
                            nmm = 3 * (KO // 2)
                            i = 0
                            for xt, wt in ((xsh, wvh_sb), (xsh, wvl_sb), (xsl, wvh_sb)):
                                for j in range(KO // 2):
                                    nc.tensor.matmul(
                                        psv[:], xt[:, 2 * j:2 * j + 2, tsl],
                                        wt[:, 2 * j:2 * j + 2, :],
                                        start=(i == 0), stop=(i == nmm - 1),
                                        perf_mode=DR)
                                    i += 1
                            nc.scalar.copy(vsb[:, ts * 4 + mt, :], psv[:])
                            for _ in range(3):
                                pump_q0()
                        xs_cur = xs_nxt
                    # drain any remaining q-strip-0 attention
                    while q0_iter[0] is not None or q0_queue:
                        pump_q0()

                # ------- Phases 2+3 interleaved, strip-pipelined -------
                with tc.tile_pool(name="p2pt", bufs=16) as p2pt, \
                     tc.tile_pool(name="p2r", bufs=2) as p2r, \
                     tc.tile_pool(name="p2acc", bufs=2) as p2acc, \
                     tc.tile_pool(name="p2y", bufs=3) as p2y, \
                     tc.tile_pool(name="p3w", bufs=1) as p3w, \
                     tc.tile_pool(name="p3o", bufs=3) as p3o, \
                     tc.tile_pool(name="psS", bufs=4, space="PSUM") as psS, \
                     tc.tile_pool(name="psCB", bufs=1, space="PSUM") as psCB, \
                     tc.tile_pool(name="psO", bufs=1, space="PSUM") as psO, \
                     tc.tile_pool(name="p3ps", bufs=2, space="PSUM") as p3ps:
                    pools = dict(psS=psS, psCB=psCB, psO=psO,
                                 pt=p2pt, acc=p2acc, p2r=p2r, p2y=p2y)
                    woh_sb = p3w.tile([128, NHC, H], DT.float8e4)
                    wol_sb = p3w.tile([128, NHC, H], DT.float8e4)
                    nc.sync.dma_start(woh_sb[:], woh_r[:])
                    nc.sync.dma_start(wol_sb[:], wol_r[:])

                    ot_state = {}  # tt -> ot tile being filled

                    def oproj_group(g, qj_src):
                        # g in [0, 32): one [128,512] o_proj PSUM group of
                        # strip qj_src; stores batch 4 n-chunks per DMA.
                        tt = 4 * qj_src + g // 8
                        nt = g % 8
                        tsl = slice(tt * 128, (tt + 1) * 128)
                        n0 = nt * 512
                        ps3 = p3ps.tile([128, 512], DT.float32, tag="p3")
                        nmm = 3 * (NHC // 2)
                        i = 0
                        for yt, wt in ((yh, woh_sb), (yl, woh_sb),
                                       (yh, wol_sb)):
                            for j in range(NHC // 2):
                                nc.tensor.matmul(
                                    ps3[:],
                                    yt[:, 2 * j:2 * j + 2, tsl],
                                    wt[:, 2 * j:2 * j + 2, n0:n0 + 512],
                                    start=(i == 0), stop=(i == nmm - 1),
                                    perf_mode=DR)
                                i += 1
                        # batch 4 n-chunks per store DMA; the final strip uses
                        # quarter stores on the idle SP queue (shorter drain)
                        per = 2 if qj_src == 3 else 4
                        half, hnt = nt // per, nt % per
                        if hnt == 0:
                            ot_new = p3o.tile([128, per, 512], DT.bfloat16,
                                              tag=f"ot{per}")
                            ot_state[tt] = ot_new
                        ot = ot_state[tt]
                        with nc.allow_low_precision(reason="bf16 partial out"):
                            nc.vector.tensor_scalar_mul(
                                ot[:, hnt, :], ps3[:], 1.0 / WSCALE)
                        if hnt == per - 1:
                            eng = nc.sync if qj_src == 3 else nc.gpsimd
                            eng.dma_start(
                                outp[tsl, half * 512 * per:(half + 1) * 512 * per]
                                .rearrange("p (h n) -> p h n", h=per),
                                ot[:])

                    nqj = 4 if 2 in phases else 0
                    for qj in range(1, nqj):
                        nki = 4 * qj + 4
                        # o_proj group positions inside the ki loop, 4/head
                        slots = {(si + 1) * nki // 4 - 1: si
                                 for si in range(4)}
                        # qj=1 head 0 was hoisted into phase 1; its 4 o_proj
                        # groups run as a prefix block instead
                        h0 = 1 if qj == 1 else 0
                        if 3 in phases and qj == 1 and 1 in phases:
                            for g in range(4):
                                oproj_group(g, 0)
                        for h in range(h0, NHC):
                            it = attn_head(qj, h, pools)
                            for ki in it:
                                if 3 in phases and ki in slots:
                                    oproj_group(4 * h + slots[ki], qj - 1)
                        if qj == nqj - 1 and 3 in phases:
                            for g in range(32):
                                oproj_group(g, qj)
    nc.compile()
    return nc


def _get_nc():
    if "nc" not in _CACHE:
        _CACHE["nc"] = _build_nc()
    return _CACHE["nc"]


def _split8(a):
    hi = np.asarray(a, np.float32).astype(F8)
    lo = (np.asarray(a, np.float32) - hi.astype(np.float32)).astype(F8)
    return hi, lo


def _host_prep(hidden_states, position_ids, wq, wk, wv, wo):
    """Build the 8 per-core input maps (all DMA-friendly layouts)."""
    inv_freq = 1.0 / (ROPE_THETA ** (np.arange(0, D, 2, dtype=np.float32) / D))
    p = np.arange(128)[:, None]
    f = np.arange(128)[None, :]
    maskT = np.ascontiguousarray((p <= f).astype(BF16))

    wq = np.asarray(wq, np.float32)
    wk = np.asarray(wk, np.float32)
    wv = np.asarray(wv, np.float32)
    wo = np.asarray(wo, np.float32)

    in_maps = []
    for c in range(8):
        b, g = divmod(c, TP)
        # X^T [H, S] -> [p, ko, strip, 512] -> [strip, p, ko*512]
        xT = np.asarray(hidden_states[b], dtype=np.float32).T
        x4 = xT.reshape(KO, 128, NSTRIP, 512).transpose(2, 1, 0, 3)
        x4 = np.ascontiguousarray(x4).reshape(NSTRIP, 128, KO * 512)
        xh_, xl_ = _split8(x4)
        # Q+K weights [out, H] -> [out=10 slots of 128 dims][p][ko*128]
        wqk = np.concatenate([wq[DQ * g:DQ * (g + 1)],
                              wk[DKV * g:DKV * (g + 1)]], axis=0) * WSCALE
        wqkT = wqk.T.reshape(KO, 128, NPROJ, 128).transpose(2, 1, 0, 3)
        wqkT = np.ascontiguousarray(wqkT).reshape(NPROJ, 128, KO * 128)
        wh_, wl_ = _split8(wqkT)
        # V weights [H, DKV] -> [p, ko*DKV]
        wvT = (wv[DKV * g:DKV * (g + 1)].T * WSCALE)
        wvT = wvT.reshape(KO, 128, DKV).transpose(1, 0, 2)
        wvT = np.ascontiguousarray(wvT).reshape(128, KO * DKV)
        wvh_, wvl_ = _split8(wvT)
        # o_proj weights [DQ, H] -> [p, head*H]
        woT = (wo[:, DQ * g:DQ * (g + 1)].T * WSCALE)  # [DQ, H]
        woTb = woT.reshape(NHC, 128, H).transpose(1, 0, 2)
        woTb = np.ascontiguousarray(woTb).reshape(128, NHC * H)
        woh_, wol_ = _split8(woTb)
        pos = np.asarray(position_ids[b], dtype=np.float32)
        freqs = pos[:, None] * inv_freq[None, :]            # [S, 64]
        cos = np.cos(freqs).T                                # [64, S]
        sin = np.sin(freqs).T
        cosF = np.ascontiguousarray(
            np.concatenate([cos, cos], axis=0).astype(np.float16))
        snF = np.ascontiguousarray(
            np.concatenate([-sin, sin], axis=0).astype(np.float16))
        in_maps.append({
            "xh": xh_, "xl": xl_, "wh": wh_, "wl": wl_,
            "wvh": wvh_, "wvl": wvl_, "woh": woh_, "wol": wol_,
            "cosF": cosF, "snF": snF, "maskT": maskT,
        })
    return in_maps


def _run(in_maps, trace=False):
    from concourse.bass_utils import run_bass_kernel_spmd
    nc = _get_nc()
    return run_bass_kernel_spmd(nc, in_maps, list(range(8)))


def kernel(hidden_states, position_ids, wq, wk, wv, wo):
    hidden_states = np.asarray(hidden_states)
    in_maps = _host_prep(hidden_states, position_ids, wq, wk, wv, wo)
    res = _run(in_maps)
    out = np.zeros((DP, S, H), dtype=np.float32)
    for c in range(8):
        b = c // TP
        out[b] += res.results[c]["outp"].astype(np.float32)
    return out


# revision 61
# speedup vs baseline: 1.0094x; 1.0030x over previous
"""Mistral attention (B=2, S=2048, H=4096, 32 q heads / 8 kv heads, rope) on
8 Trainium2 NeuronCores.

Sharding: DP=2 over batch x TP=4 over head groups. Core c handles batch
c//4 and q-heads 8g..8g+7 (kv heads 2g, 2g+1) where g = c%4. Attention is
fully local per core; each core produces a partial o_proj output
[2048, 4096] (bf16, contribution of its head group) and the host sums the
four TP partials per batch in fp32.

Mixed precision (validated on hw against the fp32 reference, relmax ~3e-3):
  - All projection matmuls (Q/K/V, o_proj) run as fp8-e4m3 DoubleRow
    matmuls (contraction 256/instr at 0.5 cycles/row) with a 3-term
    residual expansion  x_hi*w_hi + x_hi*w_lo + x_lo*w_hi  where
    t_hi = fp8(t), t_lo = fp8(t - t_hi).  X and all weights are split on
    the host; the o_proj input Y is split on-device during softmax
    normalization.  Weights are pre-scaled by 64 so fp8 quantization of
    W*64 stays in the normal range; the 64s cancel via the exp scale, the
    64-valued ones-vector in the denominator reduction, and a 1/64 scale
    on the output store.
  - Attention stays >= bf16 precision: scores fp16 x fp16, P bf16,
    PV bf16 x bf16, softmax denominator accumulated in bf16 (DVE 2x).
    The reciprocal and its PE-broadcast are bf16: denominators reach ~1e7
    so the reciprocal needs bf16's exponent range (fp16 underflows).
  - RoPE in [d, t] layout: 4 DVE ops reading the fp32 projection PSUM
    directly (walrus allows partition-shifted reads only for PSUM inputs)
    with host tables cosF=[cos;cos], snF=[-sin;+sin], fp16 out.

Schedule: PE engines execute in program order, so the program is laid out
to keep PE saturated:
  - Phase 1 streams X strips + per-output weights (all DMAs are
    per-partition contiguous, >=4KB segments, issued on SP in consumption
    order with double-buffered pools).
  - The q-strip-0 attention (ACT-heavy, little PE work) is hoisted into
    phase 1, one head after each strip-2/3 projection job, where the ACT
    engine is otherwise idle.  It uses a dedicated small PSUM pool set.
  - In phase 2, each remaining q-strip's attention interleaves the
    PREVIOUS strip's o_proj PSUM groups inside the ki loop (4 per head),
    giving PE filler while ACT exps drain.  Output stores batch 4
    n-chunks per DMA and issue on the idle GPSIMD queue.
  - Scores/exp tiles on the causal diagonal are restricted to their valid
    q-span; a single [128,128] lower-triangle mask handles the partial
    window.
"""
import sys

if "/opt/trn_rl_repo" not in sys.path:
    sys.path.insert(0, "/opt/trn_rl_repo")

import numpy as np
import ml_dtypes

BF16 = ml_dtypes.bfloat16
F8 = ml_dtypes.float8_e4m3

S = 2048          # sequence length per core
H = 4096          # hidden
D = 128           # head dim
TP = 4            # head-group shards
DP = 2            # batch shards
NHC = 8           # q heads per core
NKVC = 2          # kv heads per core
NPROJ = NHC + NKVC  # 10 projection outputs of 128 dims (8 q heads + 2 kv)
DQ = NHC * D      # 1024 q-proj out dims per core
DKV = NKVC * D    # 256 kv-proj out dims per core
KO = H // 128     # 32 contraction tiles for projections
NT = S // 128     # 16 t tiles
NSTRIP = S // 512  # 4 t strips
SCALE = 1.0 / np.sqrt(D)
ROPE_THETA = 10000.0
WSCALE = 64.0     # host premultiplier on all weights (fp8 range)

_CACHE = {}


def _mybir():
    import concourse.mybir as mybir
    return mybir


def _build_nc(phases=(1, 2, 3), repeats=1, tweaks=()):
    import concourse.mybir as mybir
    import concourse.tile as tile
    from concourse import bacc

    DT = mybir.dt
    DR = mybir.MatmulPerfMode.DoubleRow
    nc = bacc.Bacc(None, target_bir_lowering=False)

    # X strips, per-partition contiguous: [strip][p][ko*512]
    xh = nc.dram_tensor("xh", [NSTRIP, 128, KO * 512], DT.float8e4, kind="ExternalInput")
    xl = nc.dram_tensor("xl", [NSTRIP, 128, KO * 512], DT.float8e4, kind="ExternalInput")
    # Q+K weights, per-output contiguous: [out][p][ko*128]
    wh = nc.dram_tensor("wh", [NPROJ, 128, KO * 128], DT.float8e4, kind="ExternalInput")
    wl = nc.dram_tensor("wl", [NPROJ, 128, KO * 128], DT.float8e4, kind="ExternalInput")
    wvh = nc.dram_tensor("wvh", [128, KO * DKV], DT.float8e4, kind="ExternalInput")
    wvl = nc.dram_tensor("wvl", [128, KO * DKV], DT.float8e4, kind="ExternalInput")
    woh = nc.dram_tensor("woh", [128, NHC * H], DT.float8e4, kind="ExternalInput")
    wol = nc.dram_tensor("wol", [128, NHC * H], DT.float8e4, kind="ExternalInput")
    cosF = nc.dram_tensor("cosF", [D, S], DT.float16, kind="ExternalInput")
    snF = nc.dram_tensor("snF", [D, S], DT.float16, kind="ExternalInput")
    maskT = nc.dram_tensor("maskT", [128, 128], DT.bfloat16, kind="ExternalInput")
    outp = nc.dram_tensor("outp", [S, H], DT.bfloat16, kind="ExternalOutput")

    xh_r = xh.rearrange("s p (ko t) -> s p ko t", ko=KO)      # [4, 128, 32, 512]
    xl_r = xl.rearrange("s p (ko t) -> s p ko t", ko=KO)
    wh_r = wh.rearrange("o p (ko m) -> o p ko m", ko=KO)      # [10, 128, 32, 128]
    wl_r = wl.rearrange("o p (ko m) -> o p ko m", ko=KO)
    wvh_r = wvh.rearrange("p (ko m) -> p ko m", ko=KO)        # [128, 32, 256]
    wvl_r = wvl.rearrange("p (ko m) -> p ko m", ko=KO)
    woh_r = woh.rearrange("p (h n) -> p h n", h=NHC)          # [128, 8, 4096]
    wol_r = wol.rearrange("p (h n) -> p h n", h=NHC)

    EXP = mybir.ActivationFunctionType.Exp

    with tile.TileContext(nc) as tc:
        with tc.tile_pool(name="persist", bufs=1) as persist:
            qT = persist.tile([128, NHC, S], DT.float16)    # [d, head, t]
            kT = persist.tile([128, NKVC, S], DT.float16)   # [d, kv, t]
            vsb = persist.tile([128, NT, DKV], DT.bfloat16)  # [t%128, ttile, dv]
            yh = persist.tile([128, NHC, S], DT.float8e4)   # [d, head, q] hi
            yl = persist.tile([128, NHC, S], DT.float8e4)   # [d, head, q] lo
            mask_sb = persist.tile([128, 128], DT.bfloat16)
            ones = persist.tile([128, 1], DT.bfloat16)
            nc.vector.memset(ones[:], WSCALE)
            ones1 = persist.tile([1, 128], DT.bfloat16)
            nc.vector.memset(ones1[:], 1.0)

            def attn_head(qj, h, P):
                """One head's attention for q-strip qj: scores + exp + mask +
                denominator + PV + normalize + fp8 y split.  P = pool dict."""
                q0 = qj * 512
                nki = 4 * qj + 4
                kv = h // (NHC // NKVC)
                # cs (denominator) and bc (broadcast) have serialized
                # lifetimes: share one PSUM bank (cs = partition-0 row)
                ps_cb = P["psCB"].tile([128, 512], DT.float32, tag="cb")
                ps_cs = ps_cb[0:1, :]
                ps_o = P["psO"].tile([128, 512], DT.float32, tag="o")
                acc = P["acc"].tile([128, 512], DT.bfloat16, tag="acc")
                for ki in range(nki):
                    k0 = ki * 128
                    r = ki - 4 * qj  # >=0 on the causal diagonal
                    v0 = max(r, 0) * 128  # valid q-span start
                    w_ = 512 - v0
                    ps_s = P["psS"].tile([128, 512], DT.float32, tag="s")
                    nc.tensor.matmul(
                        ps_s[:, 0:w_], kT[:, kv, k0:k0 + 128],
                        qT[:, h, q0 + v0:q0 + 512],
                        start=True, stop=True)
                    pt = P["pt"].tile([128, 512], DT.bfloat16, tag="pt")
                    nc.scalar.activation(pt[:, 0:w_], ps_s[:, 0:w_], EXP,
                                         scale=SCALE / (WSCALE * WSCALE))
                    if r >= 0:
                        # triangular window: cols [v0, v0+128)
                        nc.vector.tensor_mul(pt[:, 0:128], pt[:, 0:128],
                                             mask_sb[:])
                    if ki == 0:
                        nc.vector.tensor_copy(acc[:], pt[:])
                    else:
                        nc.vector.tensor_add(acc[:, v0:512], acc[:, v0:512],
                                             pt[:, 0:w_])
                    nc.tensor.matmul(
                        ps_o[:, v0:512],
                        vsb[:, ki, kv * 128:(kv + 1) * 128], pt[:, 0:w_],
                        start=(ki == 0), stop=(ki == nki - 1))
                    yield ki  # interleave point for the caller
                # denominator: 64 * sum_k pt via ones-matmul
                nc.tensor.matmul(ps_cs, ones[:], acc[:], start=True,
                                 stop=True)
                # recip values go down to ~1e-8 (big exp sums): needs bf16's
                # exponent range (fp16 underflows)
                recip = P["p2r"].tile([1, 512], DT.bfloat16, tag="recip")
                with nc.allow_low_precision(reason="bf16 softmax recip"):
                    nc.vector.reciprocal(recip[:], ps_cs)
                # broadcast recip across partitions via K=1 matmul, reusing
                # the cs bank (recip already consumed it)
                ps_bc = ps_cb
                nc.tensor.matmul(ps_bc[:], ones1[:], recip[:], start=True,
                                 stop=True)
                # early PSUM evac in fp32 (ps_o is the UNnormalized sum of
                # exps -- huge; fp16 would overflow), then normalize + split
                yun = P["p2y"].tile([128, 512], DT.float32, tag="yun")
                nc.vector.tensor_copy(yun[:], ps_o[:])
                ytmp = P["p2y"].tile([128, 512], DT.float16, tag="ytmp")
                nc.vector.tensor_mul(ytmp[:], yun[:], ps_bc[:])
                nc.scalar.copy(yh[:, h, q0:q0 + 512], ytmp[:])
                with nc.allow_low_precision(reason="fp8 residual"):
                    nc.vector.tensor_sub(yl[:, h, q0:q0 + 512], ytmp[:],
                                         yh[:, h, q0:q0 + 512])

            for _rep in range(repeats):
                # ---------------- Phase 1: projections + rope ----------------
                with tc.tile_pool(name="p1c", bufs=1) as p1c, \
                     tc.tile_pool(name="p1x", bufs=2) as p1x, \
                     tc.tile_pool(name="p1w", bufs=2) as p1w, \
                     tc.tile_pool(name="p1t", bufs=1) as p1t, \
                     tc.tile_pool(name="q0pt", bufs=8) as q0pt, \
                     tc.tile_pool(name="q0acc", bufs=2) as q0acc, \
                     tc.tile_pool(name="q0r", bufs=2) as q0r, \
                     tc.tile_pool(name="q0y", bufs=2) as q0y, \
                     tc.tile_pool(name="p1ps", bufs=2, space="PSUM") as p1ps, \
                     tc.tile_pool(name="p1pv", bufs=1, space="PSUM") as p1pv, \
                     tc.tile_pool(name="q0S", bufs=3, space="PSUM") as q0S, \
                     tc.tile_pool(name="q0CB", bufs=1, space="PSUM") as q0CB, \
                     tc.tile_pool(name="q0O", bufs=1, space="PSUM") as q0O:
                    q0pools = dict(psS=q0S, psCB=q0CB, psO=q0O,
                                   pt=q0pt, acc=q0acc, p2r=q0r, p2y=q0y)
                    cos_sb = p1c.tile([D, S], DT.float16)
                    sn_sb = p1c.tile([D, S], DT.float16)
                    wvh_sb = p1c.tile([128, KO, DKV], DT.float8e4)
                    wvl_sb = p1c.tile([128, KO, DKV], DT.float8e4)

                    def rope(dst, ps, t0):
                        # dst [128, 512] fp16 (partition = d within head);
                        # rotate_half across the partition halves:
                        #   dst = ps*cosF + swap64(ps)*snF
                        # with cosF = [cos;cos], snF = [-sin;+sin].  The
                        # partition-shifted reads keep ps in PSUM (walrus
                        # forbids mismatched base partitions for SBUF+SBUF
                        # TensorTensor inputs, but PSUM+SBUF is fine).
                        u = p1t.tile([128, 512], DT.float16, tag="ru")
                        c = p1t.tile([128, 512], DT.float16, tag="rc")
                        nc.vector.tensor_mul(u[0:64, :], ps[64:128, :],
                                             sn_sb[0:64, t0:t0 + 512])
                        nc.vector.tensor_mul(u[64:128, :], ps[0:64, :],
                                             sn_sb[64:128, t0:t0 + 512])
                        nc.vector.tensor_mul(c[:], ps[:],
                                             cos_sb[:, t0:t0 + 512])
                        nc.vector.tensor_add(dst[:], c[:], u[:])

                    def load_w(h):
                        wth = p1w.tile([128, KO, 128], DT.float8e4, tag="wh")
                        wtl = p1w.tile([128, KO, 128], DT.float8e4, tag="wl")
                        nc.sync.dma_start(wth[:], wh_r[h])
                        nc.sync.dma_start(wtl[:], wl_r[h])
                        return wth, wtl

                    def load_x(ts):
                        xsh = p1x.tile([128, KO, 512], DT.float8e4, tag="xsh")
                        xsl = p1x.tile([128, KO, 512], DT.float8e4, tag="xsl")
                        return xsh, xsl

                    def x_chunk(dst, src_r, ts, c, nchunks=4):
                        ko_per = KO // nchunks
                        k0 = c * ko_per
                        nc.sync.dma_start(
                            dst[:, k0:k0 + ko_per, :],
                            src_r[ts, :, k0:k0 + ko_per, :])

                    def proj_group(ps, wth, wtl, xsh, xsl):
                        # half-ktile blocks: early matmuls only need the
                        # first half of each X tensor (startup latency)
                        nmm = 3 * (KO // 2)
                        i = 0
                        for jblk in range(2):
                            for wt, xt in ((wth, xsh), (wtl, xsh), (wth, xsl)):
                                for j in range(jblk * (KO // 4),
                                               (jblk + 1) * (KO // 4)):
                                    nc.tensor.matmul(
                                        ps[:], wt[:, 2 * j:2 * j + 2, :],
                                        xt[:, 2 * j:2 * j + 2, :],
                                        start=(i == 0), stop=(i == nmm - 1),
                                        perf_mode=DR)
                                    i += 1

                    strips = list(range(NSTRIP if 1 in phases else 0))
                    xs_cur = None
                    q0_queue = []  # pending hoisted (qj, h) attention heads
                    q0_iter = [None]  # suspended attn_head generator

                    def pump_q0():
                        if q0_iter[0] is None and q0_queue:
                            qj_, h_ = q0_queue.pop(0)
                            q0_iter[0] = attn_head(qj_, h_, q0pools)
                        if q0_iter[0] is not None:
                            try:
                                next(q0_iter[0])
                            except StopIteration:
                                q0_iter[0] = None

                    for ts in strips:
                        t0 = ts * 512
                        if ts == 0:
                            # startup order follows consumption deadlines of
                            # the half-ktile-blocked matmul order below
                            wth0 = p1w.tile([128, KO, 128], DT.float8e4, tag="wh")
                            nc.sync.dma_start(wth0[:], wh_r[0])
                            xs_cur = load_x(0)
                            x_chunk(xs_cur[0], xh_r, 0, 0)
                            wtl0 = p1w.tile([128, KO, 128], DT.float8e4, tag="wl")
                            nc.sync.dma_start(wtl0[:], wl_r[0])
                            w_cur = (wth0, wtl0)
                            x_chunk(xs_cur[0], xh_r, 0, 1)
                            x_chunk(xs_cur[1], xl_r, 0, 0)
                            x_chunk(xs_cur[1], xl_r, 0, 1)
                            nc.sync.dma_start(cos_sb[:], cosF[:])
                            nc.sync.dma_start(sn_sb[:], snF[:])
                            x_chunk(xs_cur[0], xh_r, 0, 2)
                            x_chunk(xs_cur[0], xh_r, 0, 3)
                            x_chunk(xs_cur[1], xl_r, 0, 2)
                            x_chunk(xs_cur[1], xl_r, 0, 3)
                            nc.sync.dma_start(mask_sb[:], maskT[:])
                        xsh, xsl = xs_cur
                        xs_nxt = load_x(ts + 1) if ts + 1 in strips else None
                        # Q^T heads and K^T kv-heads ([d, t] layout)
                        for h in range(NPROJ):
                            wth, wtl = w_cur
                            # prefetch next output's weights (and, spread over
                            # the loop, this/next strip's X chunks + consts)
                            w_nxt = None
                            if h + 1 < NPROJ:
                                w_nxt = load_w(h + 1)
                            elif ts + 1 in strips:
                                w_nxt = load_w(0)
                            if ts == 0:
                                if h == 6:
                                    nc.sync.dma_start(wvh_sb[:], wvh_r[:])
                                elif h == 7:
                                    nc.sync.dma_start(wvl_sb[:], wvl_r[:])
                            if 2 <= h < 6 and xs_nxt is not None:
                                x_chunk(xs_nxt[0], xh_r, ts + 1, h - 2)
                            elif 6 <= h and xs_nxt is not None:
                                x_chunk(xs_nxt[1], xl_r, ts + 1, h - 6)
                            ps = p1ps.tile([128, 512], DT.float32, tag="proj")
                            proj_group(ps, wth, wtl, xsh, xsl)
                            if h < NHC:
                                rope(qT[:, h, t0:t0 + 512], ps, t0)
                            else:
                                rope(kT[:, h - NHC, t0:t0 + 512], ps, t0)
                            w_cur = w_nxt
                            # hoisted q-strip-0 attention, paced so the heads
                            # stretch into the V region (fills V-evac gaps)
                            if 2 in phases and 1 in phases:
                                if ts == 2 and h == 0:
                                    q0_queue.extend((0, hh) for hh in range(NHC))
                                for _ in range(3):
                                    pump_q0()
                        # V natural layout [t, dv]
                        if 2 in phases and 1 in phases and ts == 3:
                            # fill strip-3's V-evac gaps with q-strip-1 head 0
                            q0_queue.append((1, 0))
                        for mt in range(4):
                            tsl = slice(mt * 128, (mt + 1) * 128)
                            psv = p1pv.tile([128, DKV], DT.float32, tag="pv")
                            nmm = 3 * (KO // 2)
                            i = 0
                            for xt, wt in ((xsh, wvh_sb), (xsh, wvl_sb), (xsl, wvh_sb)):
                                for j in range(KO // 2):
                                    nc.tensor.matmul(
                                        psv[:], xt[:, 2 * j:2 * j + 2, tsl],
                                        wt[:, 2 * j:2 * j + 2, :],
                                        start=(i == 0), stop=(i == nmm - 1),
                                        perf_mode=DR)
                                    i += 1
                            nc.scalar.copy(vsb[:, ts * 4 + mt, :], psv[:])
                            for _ in range(3):
                                pump_q0()
                        xs_cur = xs_nxt
                    # drain any remaining q-strip-0 attention
                    while q0_iter[0] is not None or q0_queue:
                        pump_q0()

                # ------- Phases 2+3 interleaved, strip-pipelined -------
                with tc.tile_pool(name="p2pt", bufs=16) as p2pt, \
                     tc.tile_pool(name="p2r", bufs=2) as p2r, \
                     tc.tile_pool(name="p2acc", bufs=2) as p2acc, \
                     tc.tile_pool(name="p2y", bufs=3) as p2y, \
                     tc.tile_pool(name="p3w", bufs=1) as p3w, \
                     tc.tile_pool(name="p3o", bufs=3) as p3o, \
                     tc.tile_pool(name="psS", bufs=4, space="PSUM") as psS, \
                     tc.tile_pool(name="psCB", bufs=1, space="PSUM") as psCB, \
                     tc.tile_pool(name="psO", bufs=1, space="PSUM") as psO, \
                     tc.tile_pool(name="p3ps", bufs=2, space="PSUM") as p3ps:
                    pools = dict(psS=psS, psCB=psCB, psO=psO,
                                 pt=p2pt, acc=p2acc, p2r=p2r, p2y=p2y)
                    woh_sb = p3w.tile([128, NHC, H], DT.float8e4)
                    wol_sb = p3w.tile([128, NHC, H], DT.float8e4)
                    nc.sync.dma_start(woh_sb[:], woh_r[:])
                    nc.sync.dma_start(wol_sb[:], wol_r[:])

                    ot_state = {}  # tt -> ot tile being filled

                    def oproj_group(g, qj_src):
                        # g in [0, 32): one [128,512] o_proj PSUM group of
                        # strip qj_src; stores batch 4 n-chunks per DMA.
                        tt = 4 * qj_src + g // 8
                        nt = g % 8
                        tsl = slice(tt * 128, (tt + 1) * 128)
                        n0 = nt * 512
                        ps3 = p3ps.tile([128, 512], DT.float32, tag="p3")
                        nmm = 3 * (NHC // 2)
                        i = 0
                        for yt, wt in ((yh, woh_sb), (yl, woh_sb),
                                       (yh, wol_sb)):
                            for j in range(NHC // 2):
                                nc.tensor.matmul(
                                    ps3[:],
                                    yt[:, 2 * j:2 * j + 2, tsl],
                                    wt[:, 2 * j:2 * j + 2, n0:n0 + 512],
                                    start=(i == 0), stop=(i == nmm - 1),
                                    perf_mode=DR)
                                i += 1
                        # batch 4 n-chunks per store DMA; the final strip uses
                        # quarter stores on the idle SP queue (shorter drain)
                        per = 2 if qj_src == 3 else 4
                        half, hnt = nt // per, nt % per
                        if hnt == 0:
                            ot_new = p3o.tile([128, per, 512], DT.bfloat16,
                                              tag=f"ot{per}")
                            ot_state[tt] = ot_new
                        ot = ot_state[tt]
                        with nc.allow_low_precision(reason="bf16 partial out"):
                            nc.vector.tensor_scalar_mul(
                                ot[:, hnt, :], ps3[:], 1.0 / WSCALE)
                        if hnt == per - 1:
                            eng = nc.sync if qj_src == 3 else nc.gpsimd
                            eng.dma_start(
                                outp[tsl, half * 512 * per:(half + 1) * 512 * per]
                                .rearrange("p (h n) -> p h n", h=per),
                                ot[:])

                    nqj = 4 if 2 in phases else 0
                    for qj in range(1, nqj):
                        nki = 4 * qj + 4
                        last = qj == nqj - 1
                        # qj=1 head 0 was hoisted into phase 1; its 4 o_proj
                        # groups run as a prefix block instead
                        h0 = 1 if qj == 1 else 0
                        if 3 in phases and qj == 1 and 1 in phases:
                            for g in range(4):
                                oproj_group(g, 0)
                        g = 4 * h0
                        for h in range(h0, NHC):
                            # on the last strip, hold back 3 groups as a
                            # bridge over head 7's norm chain (below)
                            cnt = 3 if (last and h >= 5) else 4
                            slots = {(si + 1) * nki // cnt - 1: g + si
                                     for si in range(cnt)}
                            g += cnt
                            it = attn_head(qj, h, pools)
                            for ki in it:
                                if 3 in phases and ki in slots:
                                    oproj_group(slots[ki], qj - 1)
                        if last and 3 in phases:
                            for gb in range(g, 32):
                                oproj_group(gb, qj - 1)
                            for g_ in range(32):
                                oproj_group(g_, qj)
    nc.compile()
    return nc


def _get_nc():
    if "nc" not in _CACHE:
        _CACHE["nc"] = _build_nc()
    return _CACHE["nc"]


def _split8(a):
    hi = np.asarray(a, np.float32).astype(F8)
    lo = (np.asarray(a, np.float32) - hi.astype(np.float32)).astype(F8)
    return hi, lo


def _host_prep(hidden_states, position_ids, wq, wk, wv, wo):
    """Build the 8 per-core input maps (all DMA-friendly layouts)."""
    inv_freq = 1.0 / (ROPE_THETA ** (np.arange(0, D, 2, dtype=np.float32) / D))
    p = np.arange(128)[:, None]
    f = np.arange(128)[None, :]
    maskT = np.ascontiguousarray((p <= f).astype(BF16))

    wq = np.asarray(wq, np.float32)
    wk = np.asarray(wk, np.float32)
    wv = np.asarray(wv, np.float32)
    wo = np.asarray(wo, np.float32)

    in_maps = []
    for c in range(8):
        b, g = divmod(c, TP)
        # X^T [H, S] -> [p, ko, strip, 512] -> [strip, p, ko*512]
        xT = np.asarray(hidden_states[b], dtype=np.float32).T
        x4 = xT.reshape(KO, 128, NSTRIP, 512).transpose(2, 1, 0, 3)
        x4 = np.ascontiguousarray(x4).reshape(NSTRIP, 128, KO * 512)
        xh_, xl_ = _split8(x4)
        # Q+K weights [out, H] -> [out=10 slots of 128 dims][p][ko*128]
        wqk = np.concatenate([wq[DQ * g:DQ * (g + 1)],
                              wk[DKV * g:DKV * (g + 1)]], axis=0) * WSCALE
        wqkT = wqk.T.reshape(KO, 128, NPROJ, 128).transpose(2, 1, 0, 3)
        wqkT = np.ascontiguousarray(wqkT).reshape(NPROJ, 128, KO * 128)
        wh_, wl_ = _split8(wqkT)
        # V weights [H, DKV] -> [p, ko*DKV]
        wvT = (wv[DKV * g:DKV * (g + 1)].T * WSCALE)
        wvT = wvT.reshape(KO, 128, DKV).transpose(1, 0, 2)
        wvT = np.ascontiguousarray(wvT).reshape(128, KO * DKV)
        wvh_, wvl_ = _split8(wvT)
        # o_proj weights [DQ, H] -> [p, head*H]
        woT = (wo[:, DQ * g:DQ * (g + 1)].T * WSCALE)  # [DQ, H]
        woTb = woT.reshape(NHC, 128, H).transpose(1, 0, 2)
        woTb = np.ascontiguousarray(woTb).reshape(128, NHC * H)
        woh_, wol_ = _split8(woTb)
        pos = np.asarray(position_ids[b], dtype=np.float32)
        freqs = pos[:, None] * inv_freq[None, :]            # [S, 64]
        cos = np.cos(freqs).T                                # [64, S]
        sin = np.sin(freqs).T
        cosF = np.ascontiguousarray(
            np.concatenate([cos, cos], axis=0).astype(np.float16))
        snF = np.ascontiguousarray(
            np.concatenate([-sin, sin], axis=0).astype(np.float16))
        in_maps.append({
            "xh": xh_, "xl": xl_, "wh": wh_, "wl": wl_,
            "wvh": wvh_, "wvl": wvl_, "woh": woh_, "wol": wol_,
            "cosF": cosF, "snF": snF, "maskT": maskT,
        })
    return in_maps


def _run(in_maps, trace=False):
    from concourse.bass_utils import run_bass_kernel_spmd
    nc = _get_nc()
    return run_bass_kernel_spmd(nc, in_maps, list(range(8)))


def kernel(hidden_states, position_ids, wq, wk, wv, wo):
    hidden_states = np.asarray(hidden_states)
    in_maps = _host_prep(hidden_states, position_ids, wq, wk, wv, wo)
    res = _run(in_maps)
    out = np.zeros((DP, S, H), dtype=np.float32)
    for c in range(8):
        b = c // TP
        out[b] += res.results[c]["outp"].astype(np.float32)
    return out
